# revision 1
# baseline (speedup 1.0000x reference)
"""2-layer GCN (PyG GCNConv, bias=False, normalize=True) on 8 TRN2 NeuronCores.

Math: out = A @ relu(A @ X @ W1) @ W2 with A = D^{-1/2} (A_w + I) D^{-1/2}.
Since aggregation commutes with the dense weight matmul, layer 1 is computed
as (A@X)@W1 against the replicated input X (zero communication), and layer 2
as A@(h1@W2) with a single AllGather of the small per-core H2 = h1@W2 shard.

Sharding: destination nodes are block-partitioned across the 8 cores
(core c owns rows [c*N/8, (c+1)*N/8)).  Edges (+ folded-in self loops) are
grouped on the host by destination block into 128-edge tiles; on device each
tile is one indirect DMA gather of source rows plus one TensorE matmul whose
stationary operand is a host-built [128 edges x 128 dst] indicator holding the
edge normalization coefficients, PSUM-accumulated per destination block.
"""

import math

import numpy as np

N_CORES = 8
COMPUTE_DTYPE = "bf16"  # "f32" or "bf16"
DENSE_L2 = False
SPLIT_BLOCKS = [3, 3, 3, 1]  # dst-block groups per collective split (dense_l2)


# --------------------------------------------------------------------------
# host-side graph packing
# --------------------------------------------------------------------------
def _pack_graph(edge_index, edge_weight, n_nodes, n_cores, ind_np_dtype=np.float32,
                dense_l2=False):
    src = np.asarray(edge_index[0], dtype=np.int64)
    dst = np.asarray(edge_index[1], dtype=np.int64)
    w = np.asarray(edge_weight, dtype=np.float32)

    deg = np.zeros(n_nodes, dtype=np.float32)
    np.add.at(deg, dst, w)
    deg += np.float32(1.0)
    dinv = (1.0 / np.sqrt(deg)).astype(np.float32)
    norm = (dinv[src] * w * dinv[dst]).astype(np.float32)

    # fold self loops (coefficient 1/deg) in as ordinary edges
    iota = np.arange(n_nodes, dtype=np.int64)
    s_all = np.concatenate([src, iota])
    d_all = np.concatenate([dst, iota])
    v_all = np.concatenate([norm, (1.0 / deg).astype(np.float32)])

    npc = n_nodes // n_cores          # nodes per core
    nblk = (npc + 127) // 128         # dst blocks per core

    core = d_all // npc
    dloc = d_all % npc
    blk = dloc // 128
    col = dloc % 128                  # indicator column within block

    # per-(core, block) edge counts -> SPMD-shared tile structure
    counts = np.zeros((n_cores, nblk), dtype=np.int64)
    np.add.at(counts, (core, blk), 1)
    t_blocks = [max(1, int(math.ceil(counts[:, b].max() / 128.0))) for b in range(nblk)]
    tile_off = np.concatenate([[0], np.cumsum(t_blocks)]).astype(np.int64)
    tot_tiles = int(tile_off[-1])
    tot_slots = tot_tiles * 128

    idxw_list, ind_list, cnt_list = [], [], []
    chunk_slots = 8 * 128
    s = np.arange(tot_slots)
    for c in range(n_cores):
        m = core == c
        sc, bc, cc, vc = s_all[m], blk[m], col[m], v_all[m]
        order = np.argsort(bc, kind="stable")
        sc, bc, cc, vc = sc[order], bc[order], cc[order], vc[order]
        starts = np.searchsorted(bc, np.arange(nblk))
        rank = np.arange(len(bc)) - starts[bc]
        slot = tile_off[bc] * 128 + rank

        idx_slots = np.zeros(tot_slots, dtype=np.int16)
        col_slots = np.zeros(tot_slots, dtype=np.int64)
        val_slots = np.zeros(tot_slots, dtype=np.float32)
        idx_slots[slot] = sc.astype(np.int16)
        col_slots[slot] = cc
        val_slots[slot] = vc

        # mark tail pads of each block's FINAL gather chunk as -1 so the
        # gather ucode skips their descriptors; emit per-call valid counts
        cnts = []
        for b in range(nblk):
            s0 = int(tile_off[b]) * 128
            nslot = t_blocks[b] * 128
            cnt = int(counts[c, b])
            done = 0
            while done < nslot:
                cl = min(chunk_slots, nslot - done)
                is_final = done + cl >= nslot
                if is_final and cnt > done:
                    valid = cnt - done
                    idx_slots[s0 + done + valid : s0 + done + cl] = -1
                    cnts.append(valid)
                else:
                    cnts.append(cl)
                done += cl
        cnt_list.append(np.array(cnts, dtype=np.uint32)[None, :])

        ind = np.zeros((128, tot_slots), dtype=ind_np_dtype)
        ind[s % 128, (s // 128) * 128 + col_slots] = val_slots.astype(ind_np_dtype)

        idxw = np.zeros((128, tot_slots // 16), dtype=np.int16)
        idxw[s % 16, s // 16] = idx_slots
        for r in range(1, 8):
            idxw[16 * r : 16 * (r + 1)] = idxw[:16]

        idxw_list.append(idxw)
        ind_list.append(np.ascontiguousarray(ind))

    ind2_list = []
    ns_tiles = (n_nodes + 127) // 128
    # split the H2 allgather into groups of dst blocks; concatenating the
    # groups' allgather outputs yields a PERMUTED H2 row space (group-major,
    # then core-major, then local row).  Groups start on 128-multiples of the
    # permuted space iff n_cores * 128 * (blocks so far) stays 128-aligned,
    # which it always is.
    split_blocks = SPLIT_BLOCKS if (dense_l2 and SPLIT_BLOCKS) else [nblk]
    if sum(split_blocks) != nblk:
        split_blocks = [nblk]
    grp_rows = []        # rows per core of each group
    acc = 0
    for gnb in split_blocks:
        lo = acc * 128
        hi = min((acc + gnb) * 128, npc)
        grp_rows.append(hi - lo)
        acc += gnb
    grp_pos0 = np.concatenate([[0], np.cumsum([r * n_cores for r in grp_rows])])
    tot_pos = int(grp_pos0[-1])
    assert tot_pos == n_nodes

    def pos_of(n):
        c = n // npc
        l = n % npc
        g = np.zeros_like(n)
        loc = l.copy()
        acc2 = 0
        for gi, gnb in enumerate(split_blocks):
            lo, hi = acc2 * 128, min((acc2 + gnb) * 128, npc)
            mask = (l >= lo) & (l < hi)
            g[mask] = gi
            loc[mask] = l[mask] - lo
            acc2 += gnb
        base = grp_pos0[g]
        rows = np.array(grp_rows)[g]
        return base + c * rows + loc

    if dense_l2:
        # dense [128 src x 128 dst] block tiles keyed (dst_block, src_tile)
        # in PERMUTED src space: ind2[p, (d*ns_tiles+s)*128 + c] = sum of
        # norms of edges (perm-src=128*s+p) -> (dst local col c in block d)
        for c in range(n_cores):
            m = core == c
            sc, bc, cc_, vc = s_all[m], blk[m], col[m], v_all[m]
            pp = pos_of(sc)
            ind2 = np.zeros((128, nblk * ns_tiles * 128), dtype=np.float32)
            p_arr = pp % 128
            col_arr = (bc * ns_tiles + pp // 128) * 128 + cc_
            np.add.at(ind2, (p_arr, col_arr), vc)
            ind2_list.append(np.ascontiguousarray(ind2.astype(ind_np_dtype)))

    return dict(
        cnts=cnt_list,
        n_calls=len(cnt_list[0][0]),
        ns_tiles=ns_tiles,
        split_blocks=split_blocks,
        grp_rows=grp_rows,
        grp_pos0=[int(v) for v in grp_pos0],
        ind2=ind2_list,
        npc=npc,
        nblk=nblk,
        t_blocks=t_blocks,
        tile_off=tile_off,
        tot_tiles=tot_tiles,
        idxw=idxw_list,
        ind=ind_list,
    )


# --------------------------------------------------------------------------
# device kernel
# --------------------------------------------------------------------------
def _build_nc(n_nodes, f1, f2, f3, npc, nblk, t_blocks, tile_off, n_cores,
              l1_chunk=32, l2_chunk=32, compute_dtype="f32", dense_l2=False,
              split_blocks=None, grp_rows=None, grp_pos0=None, n_calls=0):
    import concourse.mybir as mybir
    import concourse.tile as tile
    from concourse import bacc
    from concourse.masks import make_identity

    f32 = mybir.dt.float32
    i16 = mybir.dt.int16
    cdt = mybir.dt.bfloat16 if compute_dtype == "bf16" else mybir.dt.float32
    tot_tiles = int(tile_off[-1])
    tot_slots = tot_tiles * 128
    kf1, kf2 = f1 // 128, f2 // 128

    nc = bacc.Bacc(num_devices=n_cores)
    x_ext = nc.declare_dram_parameter("x", [n_nodes, f1], cdt, isOutput=False)
    w1_ext = nc.declare_dram_parameter("w1", [f1, f2], cdt, isOutput=False)
    w2_ext = nc.declare_dram_parameter("w2", [f2, f3], cdt, isOutput=False)
    ind_ext = nc.declare_dram_parameter("ind", [128, tot_slots], cdt, isOutput=False)
    idx_ext = nc.declare_dram_parameter("idxw", [128, tot_slots // 16], i16, isOutput=False)
    ns_tiles = (n_nodes + 127) // 128
    cnt_ext = nc.declare_dram_parameter("cnts", [1, n_calls], mybir.dt.uint32,
                                        isOutput=False)
    if dense_l2:
        ind2_ext = nc.declare_dram_parameter(
            "ind2", [128, nblk * ns_tiles * 128], cdt, isOutput=False
        )
    out_ext = nc.declare_dram_parameter("out", [npc, f3], f32, isOutput=True)

    with tile.TileContext(nc) as tc:
        with tc.tile_pool(name="dram", bufs=1, space="DRAM") as dpool, \
             tc.tile_pool(name="const", bufs=1) as cpool, \
             tc.tile_pool(name="gbp", bufs=3) as gpool, \
             tc.tile_pool(name="work", bufs=2) as wpool, \
             tc.tile_pool(name="psagg", bufs=2, space="PSUM") as ps_agg_p, \
             tc.tile_pool(name="pstr", bufs=1, space="PSUM") as ps_tr_p, \
             tc.tile_pool(name="psc1", bufs=1, space="PSUM") as ps_c1_p, \
             tc.tile_pool(name="psh2", bufs=1, space="PSUM") as ps_h2_p, \
             tc.tile_pool(name="pso", bufs=2, space="PSUM") as ps_o_p:

            # one Pool-engine register per distinct gather size (the register
            # free-list is small; a fresh to_reg per gather exhausts it)
            _nreg_cache = {}

            def nreg(v):
                if v not in _nreg_cache:
                    _nreg_cache[v] = nc.gpsimd.to_reg(v)
                return _nreg_cache[v]

            # shared-scratchpad collective outputs are only supported for >4 cores
            h2_addr_space = "Shared" if n_cores > 4 else "Local"
            if dense_l2:
                ngrp = len(split_blocks)
                cc_in_g = [
                    dpool.tile([grp_rows[g], f3], cdt, name=f"ccin{g}")
                    for g in range(ngrp)
                ]
                h2p_g = [
                    dpool.tile([grp_rows[g] * n_cores, f3], cdt,
                               addr_space=h2_addr_space, name=f"h2p{g}")
                    for g in range(ngrp)
                ]
            else:
                cc_in = dpool.tile([npc, f3], cdt)
                h2_full = dpool.tile([n_nodes, f3], cdt, addr_space=h2_addr_space)

            idx_sb = cpool.tile([128, tot_slots // 16], i16)
            nc.sync.dma_start(out=idx_sb[:, :], in_=idx_ext[:, :])
            cnt_sb = cpool.tile([1, n_calls], mybir.dt.uint32)
            nc.sync.dma_start(out=cnt_sb[:, :], in_=cnt_ext[:, :])
            cnt_reg = nc.gpsimd.to_reg(0)
            call_i = [0]
            ind_sb = cpool.tile([128, tot_slots], cdt)
            nc.sync.dma_start(out=ind_sb[:, :], in_=ind_ext[:, :])

            w1_sb = cpool.tile([128, kf1 * f2], cdt)  # chunk (k,m) at (k*kf2+m)*128
            for k in range(kf1):
                for m_ in range(kf2):
                    nc.sync.dma_start(
                        out=w1_sb[:, (k * kf2 + m_) * 128 : (k * kf2 + m_ + 1) * 128],
                        in_=w1_ext[k * 128 : (k + 1) * 128, m_ * 128 : (m_ + 1) * 128],
                    )
            w2_sb = cpool.tile([128, kf2 * f3], cdt)
            for k in range(kf2):
                nc.sync.dma_start(
                    out=w2_sb[:, k * f3 : (k + 1) * f3],
                    in_=w2_ext[k * 128 : (k + 1) * 128, :],
                )
            ident = cpool.tile([128, 128], cdt)
            make_identity(nc, ident)

            # ---------------- layer 1 ----------------
            if dense_l2:
                grp_end = []
                acc = 0
                for gnb in split_blocks:
                    grp_end.append(acc + gnb - 1)
                    acc += gnb

                def emit_cc(g):
                    nc.gpsimd.collective_compute(
                        "AllGather",
                        mybir.AluOpType.bypass,
                        replica_groups=[list(range(n_cores))],
                        ins=[cc_in_g[g][:, :].opt()],
                        outs=[h2p_g[g][:, :].opt()],
                    )

            for b in range(nblk):
                nb = min(128, npc - b * 128)
                tb = t_blocks[b]
                tt0 = int(tile_off[b])
                ps_agg = ps_agg_p.tile([128, f1], f32, tag="agg")
                done = 0
                while done < tb:
                    ct = min(l1_chunk, tb - done)
                    gb = gpool.tile([128, ct, f1], cdt, tag="gbuf")
                    cb = (tt0 + done) * 8
                    if done + ct >= tb:
                        # final chunk of the block carries the -1 index tail;
                        # its skipped rows must read as finite zeros
                        nc.vector.memset(gb[:, :ct, :], 0.0)
                    nc.gpsimd.reg_load(
                        cnt_reg, cnt_sb[0:1, call_i[0] : call_i[0] + 1]
                    )
                    call_i[0] += 1
                    nc.gpsimd.dma_gather(
                        out_ap=gb[:, :ct, :],
                        in_ap=x_ext[:, :],
                        idxs_ap=idx_sb[:, cb : cb + ct * 8],
                        num_idxs=ct * 128,
                        num_idxs_reg=cnt_reg,
                        elem_size=f1,
                    )
                    for t in range(ct):
                        tt = tt0 + done + t
                        nc.tensor.matmul(
                            ps_agg[:, :],
                            lhsT=ind_sb[:, tt * 128 : (tt + 1) * 128],
                            rhs=gb[:, t, :],
                            start=(tt == tt0),
                            stop=(tt == tt0 + tb - 1),
                        )
                    done += ct
                if dense_l2:
                    # a group that completed at block b-1 has its h2 ready by
                    # now; dispatching here costs Pool no stall
                    for g in range(len(split_blocks)):
                        if grp_end[g] == b - 1:
                            emit_cc(g)

                agg_sb = wpool.tile([128, f1], cdt, tag="agg_sb")
                nc.vector.tensor_copy(agg_sb[:, :], ps_agg[:, :])
                ps_tr = ps_tr_p.tile([128, f1], cdt, tag="tr")
                for k in range(kf1):
                    nc.tensor.transpose(
                        ps_tr[:, k * 128 : (k + 1) * 128],
                        agg_sb[:, k * 128 : (k + 1) * 128],
                        ident,
                    )
                aggT_sb = wpool.tile([128, f1], cdt, tag="aggT")
                nc.vector.tensor_copy(aggT_sb[:, :], ps_tr[:, :])

                ps_c1 = ps_c1_p.tile([128, f2], f32, tag="c1")
                firstmm = True
                for m_ in range(kf2):
                    for k in range(kf1):
                        nc.tensor.matmul(
                            ps_c1[:, m_ * 128 : (m_ + 1) * 128],
                            lhsT=w1_sb[:, (k * kf2 + m_) * 128 : (k * kf2 + m_ + 1) * 128],
                            rhs=aggT_sb[:, k * 128 : (k + 1) * 128],
                            start=firstmm,
                            stop=(m_ == kf2 - 1 and k == kf1 - 1),
                        )
                        firstmm = False
                h1T_sb = wpool.tile([128, f2], cdt, tag="h1T")
                nc.scalar.activation(
                    h1T_sb[:, :], ps_c1[:, :], mybir.ActivationFunctionType.Relu
                )
                ps_h2 = ps_h2_p.tile([128, f3], f32, tag="h2")
                for k in range(kf2):
                    nc.tensor.matmul(
                        ps_h2[:, :],
                        lhsT=h1T_sb[:, k * 128 : (k + 1) * 128],
                        rhs=w2_sb[:, k * f3 : (k + 1) * f3],
                        start=(k == 0),
                        stop=(k == kf2 - 1),
                    )
                h2_sb = wpool.tile([128, f3], cdt, tag="h2sb")
                nc.scalar.copy(h2_sb[:, :], ps_h2[:, :])
                if dense_l2:
                    acc = 0
                    for g, gnb in enumerate(split_blocks):
                        if b < acc + gnb:
                            off = (b - acc) * 128
                            nc.sync.dma_start(
                                out=cc_in_g[g][off : off + nb, :], in_=h2_sb[:nb, :]
                            )
                            break
                        acc += gnb
                else:
                    nc.sync.dma_start(
                        out=cc_in[b * 128 : b * 128 + nb, :], in_=h2_sb[:nb, :]
                    )

            # ---------------- allgather of H2 ----------------
            if dense_l2:
                for g in range(len(split_blocks)):
                    if grp_end[g] == nblk - 1:
                        emit_cc(g)
            else:
                nc.gpsimd.collective_compute(
                    "AllGather",
                    mybir.AluOpType.bypass,
                    replica_groups=[list(range(n_cores))],
                    ins=[cc_in[:, :].opt()],
                    outs=[h2_full[:, :].opt()],
                )

            # ---------------- layer 2 ----------------
            if dense_l2:
                # per-group SBUF H2 tiles in permuted [p, src_tile, f] layout
                grp_tile0 = [p0 // 128 for p0 in grp_pos0]   # first src tile of group
                h2f_g = []
                for g in range(ngrp):
                    gpos = grp_rows[g] * n_cores
                    gt = (gpos + 127) // 128
                    h2f = cpool.tile([128, gt, f3], cdt, name=f"h2f{g}")
                    fullt = gpos // 128
                    rem = gpos - fullt * 128
                    h2v = h2p_g[g][: fullt * 128, :].rearrange(
                        "(s p) f -> p s f", p=128
                    )
                    nc.sync.dma_start(out=h2f[:, :fullt, :], in_=h2v)
                    if rem:
                        nc.vector.memset(h2f[:, fullt, :], 0.0)
                        nc.sync.dma_start(
                            out=h2f[:rem, fullt, :], in_=h2p_g[g][fullt * 128 :, :]
                        )
                    h2f_g.append(h2f)

                part_sb = {}
                for g in range(ngrp):
                    gpos = grp_rows[g] * n_cores
                    gt = (gpos + 127) // 128
                    s0 = grp_tile0[g]
                    for b in range(nblk):
                        nb = min(128, npc - b * 128)
                        i2 = gpool.tile([128, gt * 128], cdt, tag="i2", bufs=2)
                        nc.scalar.dma_start(
                            out=i2[:, :],
                            in_=ind2_ext[
                                :,
                                (b * ns_tiles + s0) * 128 : (b * ns_tiles + s0 + gt)
                                * 128,
                            ],
                        )
                        ps_o = ps_o_p.tile([128, f3], f32, tag="o")
                        for s in range(gt):
                            nc.tensor.matmul(
                                ps_o[:, :],
                                lhsT=i2[:, s * 128 : (s + 1) * 128],
                                rhs=h2f_g[g][:, s, :],
                                start=(s == 0),
                                stop=(s == gt - 1),
                            )
                        if ngrp == 1:
                            o_sb = wpool.tile([128, f3], f32, tag="osb")
                            nc.scalar.copy(o_sb[:, :], ps_o[:, :])
                            nc.sync.dma_start(
                                out=out_ext[b * 128 : b * 128 + nb, :],
                                in_=o_sb[:nb, :],
                            )
                        elif g == 0:
                            pt = wpool.tile(
                                [128, f3], f32, tag=f"part{b}", bufs=1,
                                name=f"part{b}",
                            )
                            nc.scalar.copy(pt[:, :], ps_o[:, :])
                            part_sb[b] = pt
                        elif g < ngrp - 1:
                            nc.vector.tensor_tensor(
                                out=part_sb[b][:, :],
                                in0=part_sb[b][:, :],
                                in1=ps_o[:, :],
                                op=mybir.AluOpType.add,
                            )
                        else:
                            o_sb = wpool.tile([128, f3], f32, tag="osb")
                            nc.vector.tensor_tensor(
                                out=o_sb[:, :],
                                in0=part_sb[b][:, :],
                                in1=ps_o[:, :],
                                op=mybir.AluOpType.add,
                            )
                            nc.sync.dma_start(
                                out=out_ext[b * 128 : b * 128 + nb, :],
                                in_=o_sb[:nb, :],
                            )
            else:
                call_i[0] = 0
                for b in range(nblk):
                    nb = min(128, npc - b * 128)
                    tb = t_blocks[b]
                    tt0 = int(tile_off[b])
                    ps_o = ps_o_p.tile([128, f3], f32, tag="o")
                    done = 0
                    while done < tb:
                        ct = min(l2_chunk, tb - done)
                        gb2 = gpool.tile([128, ct, f3], cdt, tag="gbuf")
                        cb = (tt0 + done) * 8
                        if done + ct >= tb:
                            nc.vector.memset(gb2[:, :ct, :], 0.0)
                        nc.gpsimd.reg_load(
                            cnt_reg, cnt_sb[0:1, call_i[0] : call_i[0] + 1]
                        )
                        call_i[0] += 1
                        nc.gpsimd.dma_gather(
                            out_ap=gb2[:, :ct, :],
                            in_ap=h2_full[:, :],
                            idxs_ap=idx_sb[:, cb : cb + ct * 8],
                            num_idxs=ct * 128,
                            num_idxs_reg=cnt_reg,
                            elem_size=f3,
                        )
                        for t in range(ct):
                            tt = tt0 + done + t
                            nc.tensor.matmul(
                                ps_o[:, :],
                                lhsT=ind_sb[:, tt * 128 : (tt + 1) * 128],
                                rhs=gb2[:, t, :],
                                start=(tt == tt0),
                                stop=(tt == tt0 + tb - 1),
                            )
                        done += ct
                    o_sb = wpool.tile([128, f3], f32, tag="osb")
                    nc.scalar.copy(o_sb[:, :], ps_o[:, :])
                    nc.sync.dma_start(
                        out=out_ext[b * 128 : b * 128 + nb, :], in_=o_sb[:nb, :]
                    )

    nc.finalize()
    return nc


def _make_in_maps(x, W1, W2, g, n_cores):
    maps = []
    for c in range(n_cores):
        m = {
            "x": x,
            "w1": W1,
            "w2": W2,
            "ind": g["ind"][c],
            "idxw": g["idxw"][c],
        }
        if g["ind2"]:
            m["ind2"] = g["ind2"][c]
        m["cnts"] = g["cnts"][c]
        maps.append(m)
    return maps


def build_all(x, edge_index, edge_weight, W1, W2, n_cores=N_CORES,
              compute_dtype=COMPUTE_DTYPE, dense_l2=DENSE_L2):
    """Host packing + Bass graph for the given full inputs."""
    if compute_dtype == "bf16":
        import ml_dtypes

        np_cdt = ml_dtypes.bfloat16
    else:
        np_cdt = np.float32
    x = np.ascontiguousarray(np.asarray(x, dtype=np.float32).astype(np_cdt))
    W1 = np.ascontiguousarray(np.asarray(W1, dtype=np.float32).astype(np_cdt))
    W2 = np.ascontiguousarray(np.asarray(W2, dtype=np.float32).astype(np_cdt))
    n_nodes, f1 = x.shape
    f2, f3 = W1.shape[1], W2.shape[1]
    g = _pack_graph(edge_index, edge_weight, n_nodes, n_cores, ind_np_dtype=np_cdt,
                    dense_l2=dense_l2)
    # empirically, dma_gather with num_idxs > 1024 fails at runtime; cap at 8
    # tiles (the host-side per-call valid counts assume the same chunking)
    l1_chunk = l2_chunk = 8
    nc = _build_nc(
        n_nodes, f1, f2, f3, g["npc"], g["nblk"], g["t_blocks"], g["tile_off"],
        n_cores, compute_dtype=compute_dtype, l1_chunk=l1_chunk, l2_chunk=l2_chunk,
        dense_l2=dense_l2, split_blocks=g["split_blocks"], grp_rows=g["grp_rows"],
        grp_pos0=g["grp_pos0"], n_calls=g["n_calls"],
    )
    return nc, _make_in_maps(x, W1, W2, g, n_cores), g


def kernel(x, edge_index, edge_weight, W1, W2):
    from concourse.bass_utils import run_bass_kernel_spmd

    nc, in_maps, _ = build_all(x, edge_index, edge_weight, W1, W2)
    res = run_bass_kernel_spmd(nc, in_maps, list(range(N_CORES)))
    out = np.concatenate(
        [np.asarray(res.results[c]["out"]) for c in range(N_CORES)], axis=0
    )
    return out.astype(np.float32)



# revision 12
# speedup vs baseline: 1.5028x; 1.5028x over previous
"""2-layer GCN (PyG GCNConv, bias=False, normalize=True) on 8 TRN2 NeuronCores.

Math: out = A @ relu(A @ X @ W1) @ W2 with A = D^{-1/2} (A_w + I) D^{-1/2}.

Structure (v2):
- Nodes are re-permuted into 80 bins (8 cores x 10 blocks, 125 nodes each,
  balanced by in-degree) so every (core, block) has a near-equal edge count.
- Layer 1 aggregation A@X streams a HOST-PREGATHERED tensor GX (source rows
  of X in edge-slot order, deduped per dst block) linearly from DRAM and
  PSUM-accumulates indicator matmuls: agg = sum_t IND_t^T @ GX_t. This
  removes all Pool-engine (SWDGE dma_gather) work from layer 1, which was
  the baseline bottleneck (~8.4 ns/edge of descriptor emission).
- Per block: h2 = relu(agg @ W1) @ W2 via PE transposes + weight matmuls.
- H2 shards AllGather in GROUPS (fired as soon as a group of blocks is
  done) so the collective overlaps layer-1 compute.
- Layer 2: the first N_GATHER_BLOCKS dst blocks aggregate via Pool
  dma_gather from the allgathered H2 (Pool is otherwise idle in layer 2),
  reusing the same SBUF-resident IND tiles as layer 1; remaining blocks use
  DENSE [128 src x 128 dst] indicator tiles streamed from DRAM against the
  SBUF-resident H2F tile stack (TensorE + DMA). The two paths run on
  disjoint engines and are balanced by N_GATHER_BLOCKS.
"""

import math

import numpy as np

N_CORES = 8
COMPUTE_DTYPE = "bf16"        # "f32" or "bf16"
NBLK = 10                     # dst blocks per core (128 rows each, 125 valid)
BIN_CAP = 125                 # nodes per (core, block) bin
N_GATHER_BLOCKS = 3           # layer-2 blocks aggregated via Pool dma_gather
AG_GROUPS = [4, 3, 3]         # blocks per allgather group
L1_CHUNK = 8                  # GX tiles per stream chunk
L2_CHUNK = 8                  # gather tiles per dma_gather call (<=8: 1024 idx)
D2_CHUNK = 16                 # dense ind2 tiles per stream chunk


# --------------------------------------------------------------------------
# host-side graph packing
# --------------------------------------------------------------------------
def _pack_graph(x, edge_index, edge_weight, n_nodes, n_cores, np_cdt):
    src = np.asarray(edge_index[0], dtype=np.int64)
    dst = np.asarray(edge_index[1], dtype=np.int64)
    w = np.asarray(edge_weight, dtype=np.float32)

    deg = np.zeros(n_nodes, dtype=np.float32)
    np.add.at(deg, dst, w)
    deg += np.float32(1.0)
    dinv = (1.0 / np.sqrt(deg)).astype(np.float32)
    norm = (dinv[src] * w * dinv[dst]).astype(np.float32)

    # fold self loops (coefficient 1/deg) in as ordinary edges
    iota = np.arange(n_nodes, dtype=np.int64)
    s_all = np.concatenate([src, iota])
    d_all = np.concatenate([dst, iota])
    v_all = np.concatenate([norm, (1.0 / deg).astype(np.float32)])

    nbins = n_cores * NBLK
    assert nbins * BIN_CAP == n_nodes

    # ---- balanced node -> (core, block, col) assignment by in-edge count
    indeg = np.zeros(n_nodes, dtype=np.int64)
    np.add.at(indeg, d_all, 1)
    order = np.argsort(-indeg, kind="stable")
    import heapq

    bin_load = np.zeros(nbins, dtype=np.int64)
    bin_fill = np.zeros(nbins, dtype=np.int64)
    bin_nodes = np.full((nbins, BIN_CAP), -1, dtype=np.int64)
    heap = [(0, 0, b) for b in range(nbins)]
    heapq.heapify(heap)
    for n in order:
        while True:
            _, _, b = heapq.heappop(heap)
            if bin_fill[b] < BIN_CAP:
                break
        bin_nodes[b, bin_fill[b]] = n
        bin_fill[b] += 1
        bin_load[b] += indeg[n]
        if bin_fill[b] < BIN_CAP:
            heapq.heappush(heap, (int(bin_load[b]), int(bin_fill[b]), b))
    assert (bin_fill == BIN_CAP).all()

    node_core = np.empty(n_nodes, dtype=np.int64)
    node_blk = np.empty(n_nodes, dtype=np.int64)
    node_col = np.empty(n_nodes, dtype=np.int64)
    for b in range(nbins):
        ns = bin_nodes[b]
        node_core[ns] = b // NBLK
        node_blk[ns] = b % NBLK
        node_col[ns] = np.arange(BIN_CAP)

    # ---- allgather group layout: position of node n in the gathered H2
    g_of_blk = np.empty(NBLK, dtype=np.int64)
    gb0_arr = np.empty(NBLK, dtype=np.int64)
    gnb_arr = np.empty(NBLK, dtype=np.int64)
    base = np.zeros(len(AG_GROUPS) + 1, dtype=np.int64)
    acc = 0
    for g, gnb in enumerate(AG_GROUPS):
        base[g + 1] = base[g] + n_cores * gnb * 128
        g_of_blk[acc : acc + gnb] = g
        gb0_arr[acc : acc + gnb] = acc
        gnb_arr[acc : acc + gnb] = gnb
        acc += gnb
    assert acc == NBLK

    def pos_of(nodes):
        c = node_core[nodes]
        b = node_blk[nodes]
        col = node_col[nodes]
        g = g_of_blk[b]
        return base[g] + (c * gnb_arr[b] + (b - gb0_arr[b])) * 128 + col

    n_pos = int(base[-1])
    ns_tiles = n_pos // 128                    # src tiles in H2F
    n_groups = len(AG_GROUPS)
    pos_all = pos_of(s_all)                    # per-edge source position

    # ---- per-core per-block edge lists, deduped by (block, src).
    # Slots within a block are ordered by the allgather GROUP of the source
    # position and padded to tile boundaries per group, so each layer-2
    # dma_gather call reads exactly one group's collective output (and can
    # start as soon as that group's allgather lands).
    e_core = node_core[d_all]
    e_blk = node_blk[d_all]
    e_col = node_col[d_all]

    key = (e_core * NBLK + e_blk) * n_nodes + s_all
    uniq, inv = np.unique(key, return_inverse=True)
    u_core = uniq // (NBLK * n_nodes)
    u_blk = (uniq // n_nodes) % NBLK
    u_src = uniq % n_nodes
    u_pos = pos_of(u_src)
    u_grp = np.searchsorted(base[1:], u_pos, side="right")  # group of source

    ucnt_g = np.zeros((n_cores, NBLK, n_groups), dtype=np.int64)
    np.add.at(ucnt_g, (u_core, u_blk, u_grp), 1)
    ucnt = ucnt_g.sum(axis=2)
    # shared tile structure: tiles per (block, group) = max over cores
    t_bg = np.zeros((NBLK, n_groups), dtype=np.int64)
    for b in range(NBLK):
        for g in range(n_groups):
            t_bg[b, g] = int(math.ceil(ucnt_g[:, b, g].max() / 128.0))
    t_blocks = [int(t_bg[b].sum()) for b in range(NBLK)]
    tile_off = np.concatenate([[0], np.cumsum(t_blocks)]).astype(np.int64)
    # first tile of (block, group)
    tile_off_bg = np.zeros((NBLK, n_groups), dtype=np.int64)
    for b in range(NBLK):
        o = int(tile_off[b])
        for g in range(n_groups):
            tile_off_bg[b, g] = o
            o += int(t_bg[b, g])
    tot_tiles = int(tile_off[-1])
    tot_slots = tot_tiles * 128

    # rank of each unique entry within its (core, blk, grp); sort by
    # (core, blk, grp) -- uniq is (core, blk, src)-sorted already
    sort2 = np.lexsort((u_pos, u_grp, u_blk, u_core))
    u_core, u_blk, u_src, u_pos, u_grp = (
        u_core[sort2], u_blk[sort2], u_src[sort2], u_pos[sort2], u_grp[sort2]
    )
    inv = np.argsort(sort2, kind="stable")[inv]
    ubg_key = (u_core * NBLK + u_blk) * n_groups + u_grp
    starts = np.searchsorted(ubg_key, np.arange(n_cores * NBLK * n_groups))
    rank = np.arange(len(uniq)) - starts[ubg_key]
    u_slot = tile_off_bg[u_blk, u_grp] * 128 + rank
    e_slot = u_slot[inv]

    xc = np.ascontiguousarray(np.asarray(x, dtype=np.float32).astype(np_cdt))
    f1 = xc.shape[1]

    gx_list, ind_list, idxw_list, cnt_list = [], [], [], []
    for c in range(n_cores):
        # ---- GX: [128, tot_tiles * f1], slot s -> (partition s%128, tile s//128)
        m = u_core == c
        slots_c = u_slot[m]
        src_c = u_src[m]
        rows = np.zeros((tot_slots, f1), dtype=np_cdt)
        rows[slots_c] = xc[src_c]
        gx = np.ascontiguousarray(
            rows.reshape(tot_tiles, 128, f1).transpose(1, 0, 2).reshape(128, -1)
        )
        gx_list.append(gx)

        # ---- IND: [128, tot_slots], ind[s%128, (s//128)*128 + dcol] += coef
        em = e_core == c
        es, ec, ev = e_slot[em], e_col[em], v_all[em]
        ind = np.zeros((128, tot_slots), dtype=np.float32)
        np.add.at(ind, (es % 128, (es // 128) * 128 + ec), ev)
        ind_list.append(np.ascontiguousarray(ind.astype(np_cdt)))

        # ---- layer-2 gather indices: GROUP-RELATIVE positions of the
        # unique sources (each call reads one group's collective output)
        idx_slots = np.zeros(tot_slots, dtype=np.int16)
        pos_c = u_pos[m]
        grp_c = u_grp[m]
        idx_slots[slots_c] = (pos_c - base[grp_c]).astype(np.int16)
        cnts = []
        for b in range(N_GATHER_BLOCKS):
            for gg in range(n_groups):
                s0 = int(tile_off_bg[b, gg]) * 128
                nslot = int(t_bg[b, gg]) * 128
                cnt = int(ucnt_g[c, b, gg])
                done = 0
                while done < nslot:
                    cl = min(L2_CHUNK * 128, nslot - done)
                    is_final = done + cl >= nslot
                    if is_final and cnt > done:
                        valid = cnt - done
                        idx_slots[s0 + done + valid : s0 + done + cl] = -1
                        cnts.append(valid)
                    elif cnt <= done:
                        # sub-run fully padded (no valid entries in chunk)
                        idx_slots[s0 + done : s0 + done + cl] = -1
                        cnts.append(0)
                    else:
                        cnts.append(cl)
                    done += cl
        cnt_list.append(np.array(cnts, dtype=np.uint32)[None, :])

        s = np.arange(tot_slots)
        idxw = np.zeros((128, tot_slots // 16), dtype=np.int16)
        idxw[s % 16, s // 16] = idx_slots
        for r in range(1, 8):
            idxw[16 * r : 16 * (r + 1)] = idxw[:16]
        idxw_list.append(idxw)

    # ---- dense layer-2 indicator tiles for blocks >= N_GATHER_BLOCKS
    n_dense = NBLK - N_GATHER_BLOCKS
    ind2_list = []
    for c in range(n_cores):
        em = (e_core == c) & (e_blk >= N_GATHER_BLOCKS)
        ep, eb, ec, ev = pos_all[em], e_blk[em], e_col[em], v_all[em]
        bi = eb - N_GATHER_BLOCKS
        ind2 = np.zeros((128, n_dense * ns_tiles * 128), dtype=np.float32)
        np.add.at(ind2, (ep % 128, (bi * ns_tiles + ep // 128) * 128 + ec), ev)
        ind2_list.append(np.ascontiguousarray(ind2.astype(np_cdt)))

    return dict(
        gx=gx_list,
        ind=ind_list,
        idxw=idxw_list,
        cnts=cnt_list,
        ind2=ind2_list,
        n_calls=len(cnt_list[0][0]),
        t_blocks=t_blocks,
        tile_off=tile_off,
        t_bg=t_bg,
        tile_off_bg=tile_off_bg,
        tot_tiles=tot_tiles,
        ns_tiles=ns_tiles,
        n_pos=n_pos,
        bin_nodes=bin_nodes,
    )


# --------------------------------------------------------------------------
# device kernel
# --------------------------------------------------------------------------
def _build_nc(f1, f2, f3, t_blocks, tile_off, n_cores, n_calls,
              ns_tiles, n_pos, t_bg, tile_off_bg, compute_dtype="bf16"):
    import concourse.mybir as mybir
    import concourse.tile as tile
    from concourse import bacc
    from concourse.masks import make_identity

    f32 = mybir.dt.float32
    i16 = mybir.dt.int16
    cdt = mybir.dt.bfloat16 if compute_dtype == "bf16" else mybir.dt.float32
    tot_tiles = int(tile_off[-1])
    tot_slots = tot_tiles * 128
    kf1, kf2 = f1 // 128, f2 // 128
    n_dense = NBLK - N_GATHER_BLOCKS
    # tiles of the gather blocks stay SBUF-resident (shared by both layers)
    gtiles = int(tile_off[N_GATHER_BLOCKS])

    nc = bacc.Bacc(num_devices=n_cores)
    gx_ext = nc.declare_dram_parameter("gx", [128, tot_tiles * f1], cdt, isOutput=False)
    ind_ext = nc.declare_dram_parameter("ind", [128, tot_slots], cdt, isOutput=False)
    w1_ext = nc.declare_dram_parameter("w1", [f1, f2], cdt, isOutput=False)
    w2_ext = nc.declare_dram_parameter("w2", [f2, f3], cdt, isOutput=False)
    idx_ext = nc.declare_dram_parameter("idxw", [128, tot_slots // 16], i16, isOutput=False)
    cnt_ext = nc.declare_dram_parameter("cnts", [1, max(1, n_calls)], mybir.dt.uint32,
                                        isOutput=False)
    if n_dense:
        ind2_ext = nc.declare_dram_parameter(
            "ind2", [128, n_dense * ns_tiles * 128], cdt, isOutput=False
        )
    out_ext = nc.declare_dram_parameter("out", [NBLK * 128, f3], f32, isOutput=True)

    with tile.TileContext(nc) as tc:
        with tc.tile_pool(name="dram", bufs=1, space="DRAM") as dpool, \
             tc.tile_pool(name="const", bufs=1) as cpool, \
             tc.tile_pool(name="gxp", bufs=3) as gxpool, \
             tc.tile_pool(name="indp", bufs=3) as indpool, \
             tc.tile_pool(name="i2p", bufs=2) as i2pool, \
             tc.tile_pool(name="gbp", bufs=10) as gbpool, \
             tc.tile_pool(name="work", bufs=2) as wpool, \
             tc.tile_pool(name="psagg", bufs=2, space="PSUM") as ps_agg_p, \
             tc.tile_pool(name="pstr", bufs=1, space="PSUM") as ps_tr_p, \
             tc.tile_pool(name="psc1", bufs=1, space="PSUM") as ps_c1_p, \
             tc.tile_pool(name="psh2", bufs=1, space="PSUM") as ps_h2_p, \
             tc.tile_pool(name="pso", bufs=2, space="PSUM") as ps_o_p:

            # ---- DRAM collective buffers (one Shared output per group: a
            # Shared DRAM tensor may only have a single writing instruction)
            cc_in_g = [
                dpool.tile([gnb * 128, f3], cdt, name=f"ccin{g}")
                for g, gnb in enumerate(AG_GROUPS)
            ]
            h2p_g = [
                dpool.tile([n_cores * gnb * 128, f3], cdt, addr_space="Shared",
                           name=f"h2p{g}")
                for g, gnb in enumerate(AG_GROUPS)
            ]
            grp_lo = [0]
            for gnb in AG_GROUPS:
                grp_lo.append(grp_lo[-1] + n_cores * gnb * 128)

            # ---- constants
            cnt_sb = cpool.tile([1, max(1, n_calls)], mybir.dt.uint32)
            nc.sync.dma_start(out=cnt_sb[:, :], in_=cnt_ext[:, :])
            idx_sb = cpool.tile([128, tot_slots // 16], i16)
            nc.sync.dma_start(out=idx_sb[:, :], in_=idx_ext[:, :])
            cnt_reg = nc.gpsimd.to_reg(0)
            call_i = [0]

            # IND tiles of the gather blocks: resident, used by both layers
            indg_sb = cpool.tile([128, gtiles * 128], cdt)
            nc.scalar.dma_start(out=indg_sb[:, :], in_=ind_ext[:, : gtiles * 128])

            w1_sb = cpool.tile([128, kf1 * f2], cdt)  # chunk (k,m) at (k*kf2+m)*128
            for k in range(kf1):
                for m_ in range(kf2):
                    nc.scalar.dma_start(
                        out=w1_sb[:, (k * kf2 + m_) * 128 : (k * kf2 + m_ + 1) * 128],
                        in_=w1_ext[k * 128 : (k + 1) * 128, m_ * 128 : (m_ + 1) * 128],
                    )
            w2_sb = cpool.tile([128, kf2 * f3], cdt)
            for k in range(kf2):
                nc.scalar.dma_start(
                    out=w2_sb[:, k * f3 : (k + 1) * f3],
                    in_=w2_ext[k * 128 : (k + 1) * 128, :],
                )
            ident = cpool.tile([128, 128], cdt)
            make_identity(nc, ident)

            h2f = cpool.tile([128, ns_tiles, f3], cdt, name="h2f")

            # ---- allgather group bookkeeping
            g_of_blk, gb0 = [], []
            acc = 0
            for g, gnb in enumerate(AG_GROUPS):
                for _ in range(gnb):
                    g_of_blk.append(g)
                    gb0.append(acc)
                acc += gnb

            def emit_ag(g):
                nc.gpsimd.collective_compute(
                    "AllGather",
                    mybir.AluOpType.bypass,
                    replica_groups=[list(range(n_cores))],
                    ins=[cc_in_g[g][:, :].opt()],
                    outs=[h2p_g[g][:, :].opt()],
                )
                # land the group into the SBUF-resident H2F tile stack
                t0 = grp_lo[g] // 128
                nt = (grp_lo[g + 1] - grp_lo[g]) // 128
                h2v = h2p_g[g][:, :].rearrange("(s p) f -> p s f", p=128)
                nc.scalar.dma_start(out=h2f[:, t0 : t0 + nt, :], in_=h2v)

            # ---------------- layer 1 ----------------
            for b in range(NBLK):
                tb = t_blocks[b]
                tt0 = int(tile_off[b])
                in_resident = b < N_GATHER_BLOCKS
                ps_agg = ps_agg_p.tile([128, f1], f32, tag="agg")
                done = 0
                while done < tb:
                    ct = min(L1_CHUNK, tb - done)
                    t0 = tt0 + done
                    gxb = gxpool.tile([128, ct, f1], cdt, tag="gx")
                    nc.sync.dma_start(
                        out=gxb[:, :, :],
                        in_=gx_ext[:, t0 * f1 : (t0 + ct) * f1].rearrange(
                            "p (t f) -> p t f", t=ct
                        ),
                    )
                    if in_resident:
                        indb = indg_sb[:, t0 * 128 : (t0 + ct) * 128]
                    else:
                        indb = indpool.tile([128, ct * 128], cdt, tag="ind")
                        nc.scalar.dma_start(
                            out=indb[:, :], in_=ind_ext[:, t0 * 128 : (t0 + ct) * 128]
                        )
                    for t in range(ct):
                        tt = t0 + t
                        nc.tensor.matmul(
                            ps_agg[:, :],
                            lhsT=indb[:, t * 128 : (t + 1) * 128],
                            rhs=gxb[:, t, :],
                            start=(tt == tt0),
                            stop=(tt == tt0 + tb - 1),
                        )
                    done += ct

                # ---- per-block transform h2 = relu(agg @ W1) @ W2
                agg_sb = wpool.tile([128, f1], cdt, tag="agg_sb")
                nc.vector.tensor_copy(agg_sb[:, :], ps_agg[:, :])
                ps_tr = ps_tr_p.tile([128, f1], cdt, tag="tr")
                for k in range(kf1):
                    nc.tensor.transpose(
                        ps_tr[:, k * 128 : (k + 1) * 128],
                        agg_sb[:, k * 128 : (k + 1) * 128],
                        ident,
                    )
                aggT_sb = wpool.tile([128, f1], cdt, tag="aggT")
                nc.vector.tensor_copy(aggT_sb[:, :], ps_tr[:, :])

                ps_c1 = ps_c1_p.tile([128, f2], f32, tag="c1")
                firstmm = True
                for m_ in range(kf2):
                    for k in range(kf1):
                        nc.tensor.matmul(
                            ps_c1[:, m_ * 128 : (m_ + 1) * 128],
                            lhsT=w1_sb[:, (k * kf2 + m_) * 128 : (k * kf2 + m_ + 1) * 128],
                            rhs=aggT_sb[:, k * 128 : (k + 1) * 128],
                            start=firstmm,
                            stop=(m_ == kf2 - 1 and k == kf1 - 1),
                        )
                        firstmm = False
                h1T_sb = wpool.tile([128, f2], cdt, tag="h1T")
                nc.scalar.activation(
                    h1T_sb[:, :], ps_c1[:, :], mybir.ActivationFunctionType.Relu
                )
                ps_h2 = ps_h2_p.tile([128, f3], f32, tag="h2")
                for k in range(kf2):
                    nc.tensor.matmul(
                        ps_h2[:, :],
                        lhsT=h1T_sb[:, k * 128 : (k + 1) * 128],
                        rhs=w2_sb[:, k * f3 : (k + 1) * f3],
                        start=(k == 0),
                        stop=(k == kf2 - 1),
                    )
                h2_sb = wpool.tile([128, f3], cdt, tag="h2sb")
                nc.scalar.copy(h2_sb[:, :], ps_h2[:, :])
                g = g_of_blk[b]
                off = (b - gb0[b]) * 128
                nc.sync.dma_start(
                    out=cc_in_g[g][off : off + 128, :], in_=h2_sb[:, :]
                )
                if b == gb0[b] + AG_GROUPS[g] - 1:
                    emit_ag(g)

            # ---------------- layer 2 ----------------
            # dense blocks first (their early src tiles only need early groups)
            for bi in range(n_dense):
                b = N_GATHER_BLOCKS + bi
                ps_o = ps_o_p.tile([128, f3], f32, tag="o")
                done = 0
                while done < ns_tiles:
                    ct = min(D2_CHUNK, ns_tiles - done)
                    i2 = i2pool.tile([128, ct * 128], cdt, tag="i2")
                    nc.scalar.dma_start(
                        out=i2[:, :],
                        in_=ind2_ext[
                            :, (bi * ns_tiles + done) * 128 : (bi * ns_tiles + done + ct) * 128
                        ],
                    )
                    for s in range(ct):
                        nc.tensor.matmul(
                            ps_o[:, :],
                            lhsT=i2[:, s * 128 : (s + 1) * 128],
                            rhs=h2f[:, done + s, :],
                            start=(done + s == 0),
                            stop=(done + s == ns_tiles - 1),
                        )
                    done += ct
                o_sb = wpool.tile([128, f3], f32, tag="osb")
                nc.scalar.copy(o_sb[:, :], ps_o[:, :])
                nc.sync.dma_start(
                    out=out_ext[b * 128 : (b + 1) * 128, :], in_=o_sb[:, :]
                )

            # gather blocks (Pool engine; runs concurrently with dense DMA/TE;
            # each (block, group) sub-run gathers from that group's output and
            # can start as soon as its allgather lands)
            n_groups = len(AG_GROUPS)
            for b in range(N_GATHER_BLOCKS):
                tb = t_blocks[b]
                tt0 = int(tile_off[b])
                ps_o = ps_o_p.tile([128, f3], f32, tag="o")
                for gg in range(n_groups):
                    tbg = int(t_bg[b, gg])
                    tg0 = int(tile_off_bg[b, gg])
                    done = 0
                    while done < tbg:
                        ct = min(L2_CHUNK, tbg - done)
                        gb2 = gbpool.tile([128, ct, f3], cdt, tag="gbuf")
                        cb = (tg0 + done) * 8
                        if done + ct >= tbg:
                            # final chunk of the sub-run carries the -1 index
                            # tail; skipped rows must read as finite zeros
                            nc.vector.memset(gb2[:, :ct, :], 0.0)
                        nc.gpsimd.reg_load(
                            cnt_reg, cnt_sb[0:1, call_i[0] : call_i[0] + 1]
                        )
                        call_i[0] += 1
                        nc.gpsimd.dma_gather(
                            out_ap=gb2[:, :ct, :],
                            in_ap=h2p_g[gg][:, :],
                            idxs_ap=idx_sb[:, cb : cb + ct * 8],
                            num_idxs=ct * 128,
                            num_idxs_reg=cnt_reg,
                            elem_size=f3,
                        )
                        for t in range(ct):
                            tt = tg0 + done + t
                            nc.tensor.matmul(
                                ps_o[:, :],
                                lhsT=indg_sb[:, tt * 128 : (tt + 1) * 128],
                                rhs=gb2[:, t, :],
                                start=(tt == tt0),
                                stop=(tt == tt0 + tb - 1),
                            )
                        done += ct
                o_sb = wpool.tile([128, f3], f32, tag="osb")
                nc.scalar.copy(o_sb[:, :], ps_o[:, :])
                nc.sync.dma_start(
                    out=out_ext[b * 128 : (b + 1) * 128, :], in_=o_sb[:, :]
                )

    nc.finalize()
    return nc


# --------------------------------------------------------------------------
# top level
# --------------------------------------------------------------------------
def build_all(x, edge_index, edge_weight, W1, W2, n_cores=N_CORES,
              compute_dtype=COMPUTE_DTYPE):
    if compute_dtype == "bf16":
        import ml_dtypes

        np_cdt = ml_dtypes.bfloat16
    else:
        np_cdt = np.float32
    W1c = np.ascontiguousarray(np.asarray(W1, dtype=np.float32).astype(np_cdt))
    W2c = np.ascontiguousarray(np.asarray(W2, dtype=np.float32).astype(np_cdt))
    n_nodes = np.asarray(x).shape[0]
    f1, f2, f3 = W1c.shape[0], W1c.shape[1], W2c.shape[1]
    g = _pack_graph(x, edge_index, edge_weight, n_nodes, n_cores, np_cdt)
    nc = _build_nc(
        f1, f2, f3, g["t_blocks"], g["tile_off"], n_cores, g["n_calls"],
        g["ns_tiles"], g["n_pos"], g["t_bg"], g["tile_off_bg"],
        compute_dtype=compute_dtype,
    )
    in_maps = []
    for c in range(n_cores):
        in_maps.append({
            "gx": g["gx"][c],
            "ind": g["ind"][c],
            "w1": W1c,
            "w2": W2c,
            "idxw": g["idxw"][c],
            "cnts": g["cnts"][c],
            "ind2": g["ind2"][c],
        })
    return nc, in_maps, g


def _unpermute(res, g, n_nodes, f3, n_cores):
    out = np.empty((n_nodes, f3), dtype=np.float32)
    bin_nodes = g["bin_nodes"]
    for c in range(n_cores):
        oc = np.asarray(res[c])            # [NBLK*128, f3]
        for b in range(NBLK):
            nodes = bin_nodes[c * NBLK + b]
            out[nodes] = oc[b * 128 : b * 128 + BIN_CAP]
    return out


def kernel(x, edge_index, edge_weight, W1, W2):
    from concourse.bass_utils import run_bass_kernel_spmd

    nc, in_maps, g = build_all(x, edge_index, edge_weight, W1, W2)
    res = run_bass_kernel_spmd(nc, in_maps, list(range(N_CORES)))
    outs = [res.results[c]["out"] for c in range(N_CORES)]
    return _unpermute(outs, g, np.asarray(x).shape[0], outs[0].shape[1], N_CORES)


# revision 18
# speedup vs baseline: 1.6923x; 1.1262x over previous
"""2-layer GCN (PyG GCNConv, bias=False, normalize=True) on 8 TRN2 NeuronCores.

Math: out = A @ relu(A @ X @ W1) @ W2 with A = D^{-1/2} (A_w + I) D^{-1/2}.

Structure (v2):
- Nodes are re-permuted into 80 bins (8 cores x 10 blocks, 125 nodes each,
  balanced by in-degree) so every (core, block) has a near-equal edge count.
- Layer 1 aggregation A@X streams a HOST-PREGATHERED tensor GX (source rows
  of X in edge-slot order, deduped per dst block) linearly from DRAM and
  PSUM-accumulates indicator matmuls: agg = sum_t IND_t^T @ GX_t. This
  removes all Pool-engine (SWDGE dma_gather) work from layer 1, which was
  the baseline bottleneck (~8.4 ns/edge of descriptor emission).
- Per block: h2 = relu(agg @ W1) @ W2 via PE transposes + weight matmuls.
- H2 shards AllGather in GROUPS (fired as soon as a group of blocks is
  done) so the collective overlaps layer-1 compute.
- Layer 2: the first N_GATHER_BLOCKS dst blocks aggregate via Pool
  dma_gather from the allgathered H2 (Pool is otherwise idle in layer 2),
  reusing the same SBUF-resident IND tiles as layer 1; remaining blocks use
  DENSE [128 src x 128 dst] indicator tiles streamed from DRAM against the
  SBUF-resident H2F tile stack (TensorE + DMA). The two paths run on
  disjoint engines and are balanced by N_GATHER_BLOCKS.
"""

import math

import numpy as np

N_CORES = 8
COMPUTE_DTYPE = "bf16"        # "f32" or "bf16"
NBLK = 10                     # dst blocks per core (128 rows each, 125 valid)
BIN_CAP = 125                 # nodes per (core, block) bin
N_GATHER_BLOCKS = 4           # layer-2 blocks aggregated via Pool dma_gather
AG_GROUPS = [4, 4, 2]         # blocks per allgather group
L1_CHUNK = 8                  # GX tiles per stream chunk
L2_CHUNK = 8                  # gather tiles per dma_gather call (<=8: 1024 idx)
D2_CHUNK = 16                 # dense ind2 tiles per stream chunk


# --------------------------------------------------------------------------
# host-side graph packing
# --------------------------------------------------------------------------
def _pack_graph(x, edge_index, edge_weight, n_nodes, n_cores, np_cdt):
    src = np.asarray(edge_index[0], dtype=np.int64)
    dst = np.asarray(edge_index[1], dtype=np.int64)
    w = np.asarray(edge_weight, dtype=np.float32)

    deg = np.zeros(n_nodes, dtype=np.float32)
    np.add.at(deg, dst, w)
    deg += np.float32(1.0)
    dinv = (1.0 / np.sqrt(deg)).astype(np.float32)
    norm = (dinv[src] * w * dinv[dst]).astype(np.float32)

    # fold self loops (coefficient 1/deg) in as ordinary edges
    iota = np.arange(n_nodes, dtype=np.int64)
    s_all = np.concatenate([src, iota])
    d_all = np.concatenate([dst, iota])
    v_all = np.concatenate([norm, (1.0 / deg).astype(np.float32)])

    nbins = n_cores * NBLK
    assert nbins * BIN_CAP == n_nodes

    # ---- balanced node -> (core, block, col) assignment by in-edge count
    indeg = np.zeros(n_nodes, dtype=np.int64)
    np.add.at(indeg, d_all, 1)
    order = np.argsort(-indeg, kind="stable")
    import heapq

    bin_load = np.zeros(nbins, dtype=np.int64)
    bin_fill = np.zeros(nbins, dtype=np.int64)
    bin_nodes = np.full((nbins, BIN_CAP), -1, dtype=np.int64)
    heap = [(0, 0, b) for b in range(nbins)]
    heapq.heapify(heap)
    for n in order:
        while True:
            _, _, b = heapq.heappop(heap)
            if bin_fill[b] < BIN_CAP:
                break
        bin_nodes[b, bin_fill[b]] = n
        bin_fill[b] += 1
        bin_load[b] += indeg[n]
        if bin_fill[b] < BIN_CAP:
            heapq.heappush(heap, (int(bin_load[b]), int(bin_fill[b]), b))
    assert (bin_fill == BIN_CAP).all()

    node_core = np.empty(n_nodes, dtype=np.int64)
    node_blk = np.empty(n_nodes, dtype=np.int64)
    node_col = np.empty(n_nodes, dtype=np.int64)
    for b in range(nbins):
        ns = bin_nodes[b]
        node_core[ns] = b // NBLK
        node_blk[ns] = b % NBLK
        node_col[ns] = np.arange(BIN_CAP)

    # ---- allgather group layout: position of node n in the gathered H2
    g_of_blk = np.empty(NBLK, dtype=np.int64)
    gb0_arr = np.empty(NBLK, dtype=np.int64)
    gnb_arr = np.empty(NBLK, dtype=np.int64)
    base = np.zeros(len(AG_GROUPS) + 1, dtype=np.int64)
    acc = 0
    for g, gnb in enumerate(AG_GROUPS):
        base[g + 1] = base[g] + n_cores * gnb * 128
        g_of_blk[acc : acc + gnb] = g
        gb0_arr[acc : acc + gnb] = acc
        gnb_arr[acc : acc + gnb] = gnb
        acc += gnb
    assert acc == NBLK

    def pos_of(nodes):
        c = node_core[nodes]
        b = node_blk[nodes]
        col = node_col[nodes]
        g = g_of_blk[b]
        return base[g] + (c * gnb_arr[b] + (b - gb0_arr[b])) * 128 + col

    n_pos = int(base[-1])
    ns_tiles = n_pos // 128                    # src tiles in H2F
    n_groups = len(AG_GROUPS)
    pos_all = pos_of(s_all)                    # per-edge source position

    # ---- per-core per-block edge lists, deduped by (block, src).
    # Slots within a block are ordered by the allgather GROUP of the source
    # position and padded to tile boundaries per group, so each layer-2
    # dma_gather call reads exactly one group's collective output (and can
    # start as soon as that group's allgather lands).
    e_core = node_core[d_all]
    e_blk = node_blk[d_all]
    e_col = node_col[d_all]

    key = (e_core * NBLK + e_blk) * n_nodes + s_all
    uniq, inv = np.unique(key, return_inverse=True)
    u_core = uniq // (NBLK * n_nodes)
    u_blk = (uniq // n_nodes) % NBLK
    u_src = uniq % n_nodes
    u_pos = pos_of(u_src)
    u_grp = np.searchsorted(base[1:], u_pos, side="right")  # group of source

    ucnt_g = np.zeros((n_cores, NBLK, n_groups), dtype=np.int64)
    np.add.at(ucnt_g, (u_core, u_blk, u_grp), 1)
    ucnt = ucnt_g.sum(axis=2)
    # shared tile structure: tiles per (block, group) = max over cores
    t_bg = np.zeros((NBLK, n_groups), dtype=np.int64)
    for b in range(NBLK):
        for g in range(n_groups):
            t_bg[b, g] = int(math.ceil(ucnt_g[:, b, g].max() / 128.0))
    t_blocks = [int(t_bg[b].sum()) for b in range(NBLK)]
    tile_off = np.concatenate([[0], np.cumsum(t_blocks)]).astype(np.int64)
    # first tile of (block, group)
    tile_off_bg = np.zeros((NBLK, n_groups), dtype=np.int64)
    for b in range(NBLK):
        o = int(tile_off[b])
        for g in range(n_groups):
            tile_off_bg[b, g] = o
            o += int(t_bg[b, g])
    tot_tiles = int(tile_off[-1])
    tot_slots = tot_tiles * 128

    # rank of each unique entry within its (core, blk, grp); sort by
    # (core, blk, grp) -- uniq is (core, blk, src)-sorted already
    sort2 = np.lexsort((u_pos, u_grp, u_blk, u_core))
    u_core, u_blk, u_src, u_pos, u_grp = (
        u_core[sort2], u_blk[sort2], u_src[sort2], u_pos[sort2], u_grp[sort2]
    )
    inv = np.argsort(sort2, kind="stable")[inv]
    ubg_key = (u_core * NBLK + u_blk) * n_groups + u_grp
    starts = np.searchsorted(ubg_key, np.arange(n_cores * NBLK * n_groups))
    rank = np.arange(len(uniq)) - starts[ubg_key]
    u_slot = tile_off_bg[u_blk, u_grp] * 128 + rank
    e_slot = u_slot[inv]

    xc = np.ascontiguousarray(np.asarray(x, dtype=np.float32).astype(np_cdt))
    f1 = xc.shape[1]

    gx_list, ind_list, idxw_list, cnt_list = [], [], [], []
    for c in range(n_cores):
        # ---- GX: [128, tot_tiles * f1], slot s -> (partition s%128, tile s//128)
        m = u_core == c
        slots_c = u_slot[m]
        src_c = u_src[m]
        rows = np.zeros((tot_slots, f1), dtype=np_cdt)
        rows[slots_c] = xc[src_c]
        gx = np.ascontiguousarray(
            rows.reshape(tot_tiles, 128, f1).transpose(1, 0, 2).reshape(128, -1)
        )
        gx_list.append(gx)

        # ---- IND: [128, tot_slots], ind[s%128, (s//128)*128 + dcol] += coef
        em = e_core == c
        es, ec, ev = e_slot[em], e_col[em], v_all[em]
        ind = np.zeros((128, tot_slots), dtype=np.float32)
        np.add.at(ind, (es % 128, (es // 128) * 128 + ec), ev)
        ind_list.append(np.ascontiguousarray(ind.astype(np_cdt)))

        # ---- layer-2 gather indices: GROUP-RELATIVE positions of the
        # unique sources (each call reads one group's collective output)
        idx_slots = np.zeros(tot_slots, dtype=np.int16)
        pos_c = u_pos[m]
        grp_c = u_grp[m]
        idx_slots[slots_c] = (pos_c - base[grp_c]).astype(np.int16)
        cnts = []
        # g-major order: matches the device's Pool-stream emission
        # ([AG g][gathers of group g for all blocks][AG g+1]...)
        for gg in range(n_groups):
            for b in range(N_GATHER_BLOCKS):
                s0 = int(tile_off_bg[b, gg]) * 128
                nslot = int(t_bg[b, gg]) * 128
                cnt = int(ucnt_g[c, b, gg])
                done = 0
                while done < nslot:
                    cl = min(L2_CHUNK * 128, nslot - done)
                    is_final = done + cl >= nslot
                    if is_final and cnt > done:
                        valid = cnt - done
                        idx_slots[s0 + done + valid : s0 + done + cl] = -1
                        cnts.append(valid)
                    elif cnt <= done:
                        # sub-run fully padded (no valid entries in chunk)
                        idx_slots[s0 + done : s0 + done + cl] = -1
                        cnts.append(0)
                    else:
                        cnts.append(cl)
                    done += cl
        cnt_list.append(np.array(cnts, dtype=np.uint32)[None, :])

        s = np.arange(tot_slots)
        idxw = np.zeros((128, tot_slots // 16), dtype=np.int16)
        idxw[s % 16, s // 16] = idx_slots
        for r in range(1, 8):
            idxw[16 * r : 16 * (r + 1)] = idxw[:16]
        idxw_list.append(idxw)

    # ---- dense layer-2 indicator tiles for blocks >= N_GATHER_BLOCKS
    n_dense = NBLK - N_GATHER_BLOCKS
    ind2_list = []
    for c in range(n_cores):
        em = (e_core == c) & (e_blk >= N_GATHER_BLOCKS)
        ep, eb, ec, ev = pos_all[em], e_blk[em], e_col[em], v_all[em]
        bi = eb - N_GATHER_BLOCKS
        ind2 = np.zeros((128, n_dense * ns_tiles * 128), dtype=np.float32)
        np.add.at(ind2, (ep % 128, (bi * ns_tiles + ep // 128) * 128 + ec), ev)
        ind2_list.append(np.ascontiguousarray(ind2.astype(np_cdt)))

    return dict(
        gx=gx_list,
        ind=ind_list,
        idxw=idxw_list,
        cnts=cnt_list,
        ind2=ind2_list,
        n_calls=len(cnt_list[0][0]),
        t_blocks=t_blocks,
        tile_off=tile_off,
        t_bg=t_bg,
        tile_off_bg=tile_off_bg,
        tot_tiles=tot_tiles,
        ns_tiles=ns_tiles,
        n_pos=n_pos,
        bin_nodes=bin_nodes,
    )


# --------------------------------------------------------------------------
# device kernel
# --------------------------------------------------------------------------
def _build_nc(f1, f2, f3, t_blocks, tile_off, n_cores, n_calls,
              ns_tiles, n_pos, t_bg, tile_off_bg, compute_dtype="bf16"):
    import concourse.mybir as mybir
    import concourse.tile as tile
    from concourse import bacc
    from concourse.masks import make_identity

    f32 = mybir.dt.float32
    i16 = mybir.dt.int16
    cdt = mybir.dt.bfloat16 if compute_dtype == "bf16" else mybir.dt.float32
    tot_tiles = int(tile_off[-1])
    tot_slots = tot_tiles * 128
    kf1, kf2 = f1 // 128, f2 // 128
    n_dense = NBLK - N_GATHER_BLOCKS
    # tiles of the gather blocks stay SBUF-resident (shared by both layers)
    gtiles = int(tile_off[N_GATHER_BLOCKS])

    nc = bacc.Bacc(num_devices=n_cores)
    gx_ext = nc.declare_dram_parameter("gx", [128, tot_tiles * f1], cdt, isOutput=False)
    ind_ext = nc.declare_dram_parameter("ind", [128, tot_slots], cdt, isOutput=False)
    w1_ext = nc.declare_dram_parameter("w1", [f1, f2], cdt, isOutput=False)
    w2_ext = nc.declare_dram_parameter("w2", [f2, f3], cdt, isOutput=False)
    idx_ext = nc.declare_dram_parameter("idxw", [128, tot_slots // 16], i16, isOutput=False)
    cnt_ext = nc.declare_dram_parameter("cnts", [1, max(1, n_calls)], mybir.dt.uint32,
                                        isOutput=False)
    if n_dense:
        ind2_ext = nc.declare_dram_parameter(
            "ind2", [128, n_dense * ns_tiles * 128], cdt, isOutput=False
        )
    out_ext = nc.declare_dram_parameter("out", [NBLK * 128, f3], f32, isOutput=True)

    with tile.TileContext(nc) as tc:
        with tc.tile_pool(name="dram", bufs=1, space="DRAM") as dpool, \
             tc.tile_pool(name="const", bufs=1) as cpool, \
             tc.tile_pool(name="gxp", bufs=3) as gxpool, \
             tc.tile_pool(name="indp", bufs=3) as indpool, \
             tc.tile_pool(name="i2p", bufs=2) as i2pool, \
             tc.tile_pool(name="gbp", bufs=12) as gbpool, \
             tc.tile_pool(name="work", bufs=2) as wpool, \
             tc.tile_pool(name="psagg", bufs=2, space="PSUM") as ps_agg_p, \
             tc.tile_pool(name="pstr", bufs=1, space="PSUM") as ps_tr_p, \
             tc.tile_pool(name="psc1", bufs=1, space="PSUM") as ps_c1_p, \
             tc.tile_pool(name="psh2", bufs=1, space="PSUM") as ps_h2_p, \
             tc.tile_pool(name="pso", bufs=2, space="PSUM") as ps_o_p:

            # ---- DRAM collective buffers (one Shared output per group: a
            # Shared DRAM tensor may only have a single writing instruction)
            cc_in_g = [
                dpool.tile([gnb * 128, f3], cdt, name=f"ccin{g}")
                for g, gnb in enumerate(AG_GROUPS)
            ]
            h2p_g = [
                dpool.tile([n_cores * gnb * 128, f3], cdt, addr_space="Shared",
                           name=f"h2p{g}")
                for g, gnb in enumerate(AG_GROUPS)
            ]
            grp_lo = [0]
            for gnb in AG_GROUPS:
                grp_lo.append(grp_lo[-1] + n_cores * gnb * 128)

            # ---- constants
            cnt_sb = cpool.tile([1, max(1, n_calls)], mybir.dt.uint32)
            nc.sync.dma_start(out=cnt_sb[:, :], in_=cnt_ext[:, :])
            idx_sb = cpool.tile([128, tot_slots // 16], i16)
            nc.sync.dma_start(out=idx_sb[:, :], in_=idx_ext[:, :])
            cnt_reg = nc.gpsimd.to_reg(0)
            call_i = [0]

            # IND tiles of the gather blocks: resident, used by both layers
            indg_sb = cpool.tile([128, gtiles * 128], cdt)
            nc.scalar.dma_start(out=indg_sb[:, :], in_=ind_ext[:, : gtiles * 128])

            w1_sb = cpool.tile([128, kf1 * f2], cdt)  # chunk (k,m) at (k*kf2+m)*128
            for k in range(kf1):
                for m_ in range(kf2):
                    nc.scalar.dma_start(
                        out=w1_sb[:, (k * kf2 + m_) * 128 : (k * kf2 + m_ + 1) * 128],
                        in_=w1_ext[k * 128 : (k + 1) * 128, m_ * 128 : (m_ + 1) * 128],
                    )
            w2_sb = cpool.tile([128, kf2 * f3], cdt)
            for k in range(kf2):
                nc.scalar.dma_start(
                    out=w2_sb[:, k * f3 : (k + 1) * f3],
                    in_=w2_ext[k * 128 : (k + 1) * 128, :],
                )
            ident = cpool.tile([128, 128], cdt)
            make_identity(nc, ident)

            h2f = cpool.tile([128, ns_tiles, f3], cdt, name="h2f")

            # ---- allgather group bookkeeping
            g_of_blk, gb0 = [], []
            acc = 0
            for g, gnb in enumerate(AG_GROUPS):
                for _ in range(gnb):
                    g_of_blk.append(g)
                    gb0.append(acc)
                acc += gnb

            def emit_ag(g):
                nc.gpsimd.collective_compute(
                    "AllGather",
                    mybir.AluOpType.bypass,
                    replica_groups=[list(range(n_cores))],
                    ins=[cc_in_g[g][:, :].opt()],
                    outs=[h2p_g[g][:, :].opt()],
                )
                # land the group into the SBUF-resident H2F tile stack
                t0 = grp_lo[g] // 128
                nt = (grp_lo[g + 1] - grp_lo[g]) // 128
                h2v = h2p_g[g][:, :].rearrange("(s p) f -> p s f", p=128)
                nc.scalar.dma_start(out=h2f[:, t0 : t0 + nt, :], in_=h2v)

            # ---------------- layer 1 ----------------
            # Software-pipelined: block b's aggregation matmuls are emitted
            # BEFORE block b-1's transform so the in-order PE stream never
            # stalls on the transform's vector/scalar steps.
            n_groups = len(AG_GROUPS)
            gb_tiles = {}           # (g, call#) -> (gb2 tile, ct, tg0, done)

            def emit_gathers(gg):
                # layer-2 gather sub-runs of group gg for all gather blocks;
                # dispatched on Pool right after AG(gg), matmuls emitted later
                for b in range(N_GATHER_BLOCKS):
                    tbg = int(t_bg[b, gg])
                    tg0 = int(tile_off_bg[b, gg])
                    done = 0
                    while done < tbg:
                        ct = min(L2_CHUNK, tbg - done)
                        gb2 = gbpool.tile([128, ct, f3], cdt, tag="gbuf")
                        cb = (tg0 + done) * 8
                        if done + ct >= tbg:
                            # final chunk of the sub-run carries the -1 index
                            # tail; skipped rows must read as finite zeros
                            nc.vector.memset(gb2[:, :ct, :], 0.0)
                        nc.gpsimd.reg_load(
                            cnt_reg, cnt_sb[0:1, call_i[0] : call_i[0] + 1]
                        )
                        call_i[0] += 1
                        nc.gpsimd.dma_gather(
                            out_ap=gb2[:, :ct, :],
                            in_ap=h2p_g[gg][:, :],
                            idxs_ap=idx_sb[:, cb : cb + ct * 8],
                            num_idxs=ct * 128,
                            num_idxs_reg=cnt_reg,
                            elem_size=f3,
                        )
                        gb_tiles.setdefault((b, gg), []).append((gb2, ct, tg0 + done))
                        done += ct

            def emit_agg(b):
                tb = t_blocks[b]
                tt0 = int(tile_off[b])
                in_resident = b < N_GATHER_BLOCKS
                ps_agg = ps_agg_p.tile([128, f1], f32, tag="agg")
                done = 0
                while done < tb:
                    ct = min(L1_CHUNK, tb - done)
                    t0 = tt0 + done
                    gxb = gxpool.tile([128, ct, f1], cdt, tag="gx")
                    nc.sync.dma_start(
                        out=gxb[:, :, :],
                        in_=gx_ext[:, t0 * f1 : (t0 + ct) * f1].rearrange(
                            "p (t f) -> p t f", t=ct
                        ),
                    )
                    if in_resident:
                        indb = indg_sb[:, t0 * 128 : (t0 + ct) * 128]
                    else:
                        indb = indpool.tile([128, ct * 128], cdt, tag="ind")
                        nc.scalar.dma_start(
                            out=indb[:, :], in_=ind_ext[:, t0 * 128 : (t0 + ct) * 128]
                        )
                    for t in range(ct):
                        tt = t0 + t
                        nc.tensor.matmul(
                            ps_agg[:, :],
                            lhsT=indb[:, t * 128 : (t + 1) * 128],
                            rhs=gxb[:, t, :],
                            start=(tt == tt0),
                            stop=(tt == tt0 + tb - 1),
                        )
                    done += ct
                return ps_agg

            def emit_xform(b, ps_agg):
                # h2 = relu(agg @ W1) @ W2, then stage into the collective in
                agg_sb = wpool.tile([128, f1], cdt, tag="agg_sb")
                nc.vector.tensor_copy(agg_sb[:, :], ps_agg[:, :])
                ps_tr = ps_tr_p.tile([128, f1], cdt, tag="tr")
                for k in range(kf1):
                    nc.tensor.transpose(
                        ps_tr[:, k * 128 : (k + 1) * 128],
                        agg_sb[:, k * 128 : (k + 1) * 128],
                        ident,
                    )
                aggT_sb = wpool.tile([128, f1], cdt, tag="aggT")
                nc.vector.tensor_copy(aggT_sb[:, :], ps_tr[:, :])

                ps_c1 = ps_c1_p.tile([128, f2], f32, tag="c1")
                firstmm = True
                for m_ in range(kf2):
                    for k in range(kf1):
                        nc.tensor.matmul(
                            ps_c1[:, m_ * 128 : (m_ + 1) * 128],
                            lhsT=w1_sb[:, (k * kf2 + m_) * 128 : (k * kf2 + m_ + 1) * 128],
                            rhs=aggT_sb[:, k * 128 : (k + 1) * 128],
                            start=firstmm,
                            stop=(m_ == kf2 - 1 and k == kf1 - 1),
                        )
                        firstmm = False
                h1T_sb = wpool.tile([128, f2], cdt, tag="h1T")
                nc.scalar.activation(
                    h1T_sb[:, :], ps_c1[:, :], mybir.ActivationFunctionType.Relu
                )
                ps_h2 = ps_h2_p.tile([128, f3], f32, tag="h2")
                for k in range(kf2):
                    nc.tensor.matmul(
                        ps_h2[:, :],
                        lhsT=h1T_sb[:, k * 128 : (k + 1) * 128],
                        rhs=w2_sb[:, k * f3 : (k + 1) * f3],
                        start=(k == 0),
                        stop=(k == kf2 - 1),
                    )
                h2_sb = wpool.tile([128, f3], cdt, tag="h2sb")
                nc.scalar.copy(h2_sb[:, :], ps_h2[:, :])
                g = g_of_blk[b]
                off = (b - gb0[b]) * 128
                nc.sync.dma_start(
                    out=cc_in_g[g][off : off + 128, :], in_=h2_sb[:, :]
                )
                if b == gb0[b] + AG_GROUPS[g] - 1:
                    emit_ag(g)
                    emit_gathers(g)

            pending = None
            for b in range(NBLK):
                ps_agg = emit_agg(b)
                if pending is not None:
                    emit_xform(*pending)
                pending = (b, ps_agg)
            emit_xform(*pending)

            # ---------------- layer 2 ----------------
            # dense blocks first (their early src tiles only need early groups)
            for bi in range(n_dense):
                b = N_GATHER_BLOCKS + bi
                ps_o = ps_o_p.tile([128, f3], f32, tag="o")
                done = 0
                while done < ns_tiles:
                    ct = min(D2_CHUNK, ns_tiles - done)
                    i2 = i2pool.tile([128, ct * 128], cdt, tag="i2")
                    nc.scalar.dma_start(
                        out=i2[:, :],
                        in_=ind2_ext[
                            :, (bi * ns_tiles + done) * 128 : (bi * ns_tiles + done + ct) * 128
                        ],
                    )
                    for s in range(ct):
                        nc.tensor.matmul(
                            ps_o[:, :],
                            lhsT=i2[:, s * 128 : (s + 1) * 128],
                            rhs=h2f[:, done + s, :],
                            start=(done + s == 0),
                            stop=(done + s == ns_tiles - 1),
                        )
                    done += ct
                o_sb = wpool.tile([128, f3], f32, tag="osb")
                nc.scalar.copy(o_sb[:, :], ps_o[:, :])
                nc.sync.dma_start(
                    out=out_ext[b * 128 : (b + 1) * 128, :], in_=o_sb[:, :]
                )

            # gather blocks: matmuls over the tiles dma_gather'd during the
            # allgather chain (Pool already finished or is finishing them)
            for b in range(N_GATHER_BLOCKS):
                tb = t_blocks[b]
                tt0 = int(tile_off[b])
                ps_o = ps_o_p.tile([128, f3], f32, tag="o")
                for gg in range(n_groups):
                    for gb2, ct, tstart in gb_tiles.get((b, gg), []):
                        for t in range(ct):
                            tt = tstart + t
                            nc.tensor.matmul(
                                ps_o[:, :],
                                lhsT=indg_sb[:, tt * 128 : (tt + 1) * 128],
                                rhs=gb2[:, t, :],
                                start=(tt == tt0),
                                stop=(tt == tt0 + tb - 1),
                            )
                o_sb = wpool.tile([128, f3], f32, tag="osb")
                nc.scalar.copy(o_sb[:, :], ps_o[:, :])
                nc.sync.dma_start(
                    out=out_ext[b * 128 : (b + 1) * 128, :], in_=o_sb[:, :]
                )

    nc.finalize()
    return nc


# --------------------------------------------------------------------------
# top level
# --------------------------------------------------------------------------
def build_all(x, edge_index, edge_weight, W1, W2, n_cores=N_CORES,
              compute_dtype=COMPUTE_DTYPE):
    if compute_dtype == "bf16":
        import ml_dtypes

        np_cdt = ml_dtypes.bfloat16
    else:
        np_cdt = np.float32
    W1c = np.ascontiguousarray(np.asarray(W1, dtype=np.float32).astype(np_cdt))
    W2c = np.ascontiguousarray(np.asarray(W2, dtype=np.float32).astype(np_cdt))
    n_nodes = np.asarray(x).shape[0]
    f1, f2, f3 = W1c.shape[0], W1c.shape[1], W2c.shape[1]
    g = _pack_graph(x, edge_index, edge_weight, n_nodes, n_cores, np_cdt)
    nc = _build_nc(
        f1, f2, f3, g["t_blocks"], g["tile_off"], n_cores, g["n_calls"],
        g["ns_tiles"], g["n_pos"], g["t_bg"], g["tile_off_bg"],
        compute_dtype=compute_dtype,
    )
    in_maps = []
    for c in range(n_cores):
        in_maps.append({
            "gx": g["gx"][c],
            "ind": g["ind"][c],
            "w1": W1c,
            "w2": W2c,
            "idxw": g["idxw"][c],
            "cnts": g["cnts"][c],
            "ind2": g["ind2"][c],
        })
    return nc, in_maps, g


def _unpermute(res, g, n_nodes, f3, n_cores):
    out = np.empty((n_nodes, f3), dtype=np.float32)
    bin_nodes = g["bin_nodes"]
    for c in range(n_cores):
        oc = np.asarray(res[c])            # [NBLK*128, f3]
        for b in range(NBLK):
            nodes = bin_nodes[c * NBLK + b]
            out[nodes] = oc[b * 128 : b * 128 + BIN_CAP]
    return out


def kernel(x, edge_index, edge_weight, W1, W2):
    from concourse.bass_utils import run_bass_kernel_spmd

    nc, in_maps, g = build_all(x, edge_index, edge_weight, W1, W2)
    res = run_bass_kernel_spmd(nc, in_maps, list(range(N_CORES)))
    outs = [res.results[c]["out"] for c in range(N_CORES)]
    return _unpermute(outs, g, np.asarray(x).shape[0], outs[0].shape[1], N_CORES)


# revision 23
# speedup vs baseline: 1.9055x; 1.1260x over previous
"""2-layer GCN (PyG GCNConv, bias=False, normalize=True) on 8 TRN2 NeuronCores.

Math: out = A @ relu(A @ X @ W1) @ W2 with A = D^{-1/2} (A_w + I) D^{-1/2}.

Structure (v2):
- Nodes are re-permuted into 80 bins (8 cores x 10 blocks, 125 nodes each,
  balanced by in-degree) so every (core, block) has a near-equal edge count.
- Layer 1 aggregation A@X streams a HOST-PREGATHERED tensor GX (source rows
  of X in edge-slot order, deduped per dst block) linearly from DRAM and
  PSUM-accumulates indicator matmuls: agg = sum_t IND_t^T @ GX_t. This
  removes all Pool-engine (SWDGE dma_gather) work from layer 1, which was
  the baseline bottleneck (~8.4 ns/edge of descriptor emission).
- Per block: h2 = relu(agg @ W1) @ W2 via PE transposes + weight matmuls.
- H2 shards AllGather in GROUPS (fired as soon as a group of blocks is
  done) so the collective overlaps layer-1 compute.
- Layer 2: the first N_GATHER_BLOCKS dst blocks aggregate via Pool
  dma_gather from the allgathered H2 (Pool is otherwise idle in layer 2),
  reusing the same SBUF-resident IND tiles as layer 1; remaining blocks use
  DENSE [128 src x 128 dst] indicator tiles streamed from DRAM against the
  SBUF-resident H2F tile stack (TensorE + DMA). The two paths run on
  disjoint engines and are balanced by N_GATHER_BLOCKS.
"""

import math

import numpy as np

N_CORES = 8
COMPUTE_DTYPE = "bf16"        # "f32" or "bf16"
NBLK = 10                     # dst blocks per core (128 rows each, 125 valid)
BIN_CAP = 125                 # nodes per (core, block) bin
N_GATHER_BLOCKS = 5           # layer-2 blocks aggregated via Pool dma_gather
AG_GROUPS = [4, 4, 2]         # blocks per allgather group
L1_CHUNK = 8                  # GX tiles per stream chunk
L2_CHUNK = 8                  # gather tiles per dma_gather call (<=8: 1024 idx)
D2_CHUNK = 16                 # dense ind2 tiles per stream chunk


# --------------------------------------------------------------------------
# host-side graph packing
# --------------------------------------------------------------------------
def _pack_graph(x, edge_index, edge_weight, n_nodes, n_cores, np_cdt):
    src = np.asarray(edge_index[0], dtype=np.int64)
    dst = np.asarray(edge_index[1], dtype=np.int64)
    w = np.asarray(edge_weight, dtype=np.float32)

    deg = np.zeros(n_nodes, dtype=np.float32)
    np.add.at(deg, dst, w)
    deg += np.float32(1.0)
    dinv = (1.0 / np.sqrt(deg)).astype(np.float32)
    norm = (dinv[src] * w * dinv[dst]).astype(np.float32)

    # fold self loops (coefficient 1/deg) in as ordinary edges
    iota = np.arange(n_nodes, dtype=np.int64)
    s_all = np.concatenate([src, iota])
    d_all = np.concatenate([dst, iota])
    v_all = np.concatenate([norm, (1.0 / deg).astype(np.float32)])

    nbins = n_cores * NBLK
    assert nbins * BIN_CAP == n_nodes

    # ---- balanced node -> (core, block, col) assignment by in-edge count
    indeg = np.zeros(n_nodes, dtype=np.int64)
    np.add.at(indeg, d_all, 1)
    order = np.argsort(-indeg, kind="stable")
    import heapq

    bin_load = np.zeros(nbins, dtype=np.int64)
    bin_fill = np.zeros(nbins, dtype=np.int64)
    bin_nodes = np.full((nbins, BIN_CAP), -1, dtype=np.int64)
    heap = [(0, 0, b) for b in range(nbins)]
    heapq.heapify(heap)
    for n in order:
        while True:
            _, _, b = heapq.heappop(heap)
            if bin_fill[b] < BIN_CAP:
                break
        bin_nodes[b, bin_fill[b]] = n
        bin_fill[b] += 1
        bin_load[b] += indeg[n]
        if bin_fill[b] < BIN_CAP:
            heapq.heappush(heap, (int(bin_load[b]), int(bin_fill[b]), b))
    assert (bin_fill == BIN_CAP).all()

    node_core = np.empty(n_nodes, dtype=np.int64)
    node_blk = np.empty(n_nodes, dtype=np.int64)
    node_col = np.empty(n_nodes, dtype=np.int64)
    for b in range(nbins):
        ns = bin_nodes[b]
        node_core[ns] = b // NBLK
        node_blk[ns] = b % NBLK
        node_col[ns] = np.arange(BIN_CAP)

    # ---- allgather group layout: position of node n in the gathered H2
    g_of_blk = np.empty(NBLK, dtype=np.int64)
    gb0_arr = np.empty(NBLK, dtype=np.int64)
    gnb_arr = np.empty(NBLK, dtype=np.int64)
    base = np.zeros(len(AG_GROUPS) + 1, dtype=np.int64)
    acc = 0
    for g, gnb in enumerate(AG_GROUPS):
        base[g + 1] = base[g] + n_cores * gnb * 128
        g_of_blk[acc : acc + gnb] = g
        gb0_arr[acc : acc + gnb] = acc
        gnb_arr[acc : acc + gnb] = gnb
        acc += gnb
    assert acc == NBLK

    def pos_of(nodes):
        c = node_core[nodes]
        b = node_blk[nodes]
        col = node_col[nodes]
        g = g_of_blk[b]
        return base[g] + (c * gnb_arr[b] + (b - gb0_arr[b])) * 128 + col

    n_pos = int(base[-1])
    ns_tiles = n_pos // 128                    # src tiles in H2F
    n_groups = len(AG_GROUPS)
    pos_all = pos_of(s_all)                    # per-edge source position

    # ---- per-core per-block edge lists, deduped by (block, src).
    # Slots within a block are ordered by the allgather GROUP of the source
    # position and padded to tile boundaries per group, so each layer-2
    # dma_gather call reads exactly one group's collective output (and can
    # start as soon as that group's allgather lands).
    e_core = node_core[d_all]
    e_blk = node_blk[d_all]
    e_col = node_col[d_all]

    key = (e_core * NBLK + e_blk) * n_nodes + s_all
    uniq, inv = np.unique(key, return_inverse=True)
    u_core = uniq // (NBLK * n_nodes)
    u_blk = (uniq // n_nodes) % NBLK
    u_src = uniq % n_nodes
    u_pos = pos_of(u_src)
    u_grp = np.searchsorted(base[1:], u_pos, side="right")  # group of source

    ucnt_g = np.zeros((n_cores, NBLK, n_groups), dtype=np.int64)
    np.add.at(ucnt_g, (u_core, u_blk, u_grp), 1)
    ucnt = ucnt_g.sum(axis=2)
    # shared tile structure: tiles per (block, group) = max over cores
    t_bg = np.zeros((NBLK, n_groups), dtype=np.int64)
    for b in range(NBLK):
        for g in range(n_groups):
            t_bg[b, g] = int(math.ceil(ucnt_g[:, b, g].max() / 128.0))
    t_blocks = [int(t_bg[b].sum()) for b in range(NBLK)]
    tile_off = np.concatenate([[0], np.cumsum(t_blocks)]).astype(np.int64)
    # first tile of (block, group)
    tile_off_bg = np.zeros((NBLK, n_groups), dtype=np.int64)
    for b in range(NBLK):
        o = int(tile_off[b])
        for g in range(n_groups):
            tile_off_bg[b, g] = o
            o += int(t_bg[b, g])
    tot_tiles = int(tile_off[-1])
    tot_slots = tot_tiles * 128

    # rank of each unique entry within its (core, blk, grp); sort by
    # (core, blk, grp) -- uniq is (core, blk, src)-sorted already
    sort2 = np.lexsort((u_pos, u_grp, u_blk, u_core))
    u_core, u_blk, u_src, u_pos, u_grp = (
        u_core[sort2], u_blk[sort2], u_src[sort2], u_pos[sort2], u_grp[sort2]
    )
    inv = np.argsort(sort2, kind="stable")[inv]
    ubg_key = (u_core * NBLK + u_blk) * n_groups + u_grp
    starts = np.searchsorted(ubg_key, np.arange(n_cores * NBLK * n_groups))
    rank = np.arange(len(uniq)) - starts[ubg_key]
    u_slot = tile_off_bg[u_blk, u_grp] * 128 + rank
    e_slot = u_slot[inv]

    xc = np.ascontiguousarray(np.asarray(x, dtype=np.float32).astype(np_cdt))
    f1 = xc.shape[1]

    gx_list, ind_list, idxw_list, cnt_list = [], [], [], []
    for c in range(n_cores):
        # ---- GX: [128, tot_tiles * f1], slot s -> (partition s%128, tile s//128)
        m = u_core == c
        slots_c = u_slot[m]
        src_c = u_src[m]
        rows = np.zeros((tot_slots, f1), dtype=np_cdt)
        rows[slots_c] = xc[src_c]
        gx = np.ascontiguousarray(
            rows.reshape(tot_tiles, 128, f1).transpose(1, 0, 2).reshape(128, -1)
        )
        gx_list.append(gx)

        # ---- IND: [128, tot_slots], ind[s%128, (s//128)*128 + dcol] += coef
        em = e_core == c
        es, ec, ev = e_slot[em], e_col[em], v_all[em]
        ind = np.zeros((128, tot_slots), dtype=np.float32)
        np.add.at(ind, (es % 128, (es // 128) * 128 + ec), ev)
        ind_list.append(np.ascontiguousarray(ind.astype(np_cdt)))

        # ---- layer-2 gather indices: GROUP-RELATIVE positions of the
        # unique sources (each call reads one group's collective output)
        idx_slots = np.zeros(tot_slots, dtype=np.int16)
        pos_c = u_pos[m]
        grp_c = u_grp[m]
        idx_slots[slots_c] = (pos_c - base[grp_c]).astype(np.int16)
        cnts = []
        # g-major order: matches the device's Pool-stream emission
        # ([AG g][gathers of group g for all blocks][AG g+1]...)
        for gg in range(n_groups):
            for b in range(N_GATHER_BLOCKS):
                s0 = int(tile_off_bg[b, gg]) * 128
                nslot = int(t_bg[b, gg]) * 128
                cnt = int(ucnt_g[c, b, gg])
                done = 0
                while done < nslot:
                    cl = min(L2_CHUNK * 128, nslot - done)
                    is_final = done + cl >= nslot
                    if is_final and cnt > done:
                        valid = cnt - done
                        idx_slots[s0 + done + valid : s0 + done + cl] = -1
                        cnts.append(valid)
                    elif cnt <= done:
                        # sub-run fully padded (no valid entries in chunk)
                        idx_slots[s0 + done : s0 + done + cl] = -1
                        cnts.append(0)
                    else:
                        cnts.append(cl)
                    done += cl
        cnt_list.append(np.array(cnts, dtype=np.uint32)[None, :])

        s = np.arange(tot_slots)
        idxw = np.zeros((128, tot_slots // 16), dtype=np.int16)
        idxw[s % 16, s // 16] = idx_slots
        for r in range(1, 8):
            idxw[16 * r : 16 * (r + 1)] = idxw[:16]
        idxw_list.append(idxw)

    # ---- dense layer-2 indicator tiles for blocks >= N_GATHER_BLOCKS
    n_dense = NBLK - N_GATHER_BLOCKS
    ind2_list = []
    for c in range(n_cores):
        em = (e_core == c) & (e_blk >= N_GATHER_BLOCKS)
        ep, eb, ec, ev = pos_all[em], e_blk[em], e_col[em], v_all[em]
        bi = eb - N_GATHER_BLOCKS
        ind2 = np.zeros((128, n_dense * ns_tiles * 128), dtype=np.float32)
        np.add.at(ind2, (ep % 128, (bi * ns_tiles + ep // 128) * 128 + ec), ev)
        ind2_list.append(np.ascontiguousarray(ind2.astype(np_cdt)))

    return dict(
        gx=gx_list,
        ind=ind_list,
        idxw=idxw_list,
        cnts=cnt_list,
        ind2=ind2_list,
        n_calls=len(cnt_list[0][0]),
        t_blocks=t_blocks,
        tile_off=tile_off,
        t_bg=t_bg,
        tile_off_bg=tile_off_bg,
        tot_tiles=tot_tiles,
        ns_tiles=ns_tiles,
        n_pos=n_pos,
        bin_nodes=bin_nodes,
    )


# --------------------------------------------------------------------------
# device kernel
# --------------------------------------------------------------------------
def _build_nc(f1, f2, f3, t_blocks, tile_off, n_cores, n_calls,
              ns_tiles, n_pos, t_bg, tile_off_bg, compute_dtype="bf16"):
    import concourse.mybir as mybir
    import concourse.tile as tile
    from concourse import bacc
    from concourse.masks import make_identity

    f32 = mybir.dt.float32
    i16 = mybir.dt.int16
    cdt = mybir.dt.bfloat16 if compute_dtype == "bf16" else mybir.dt.float32
    tot_tiles = int(tile_off[-1])
    tot_slots = tot_tiles * 128
    kf1, kf2 = f1 // 128, f2 // 128
    n_dense = NBLK - N_GATHER_BLOCKS
    # tiles of the gather blocks stay SBUF-resident (shared by both layers)
    gtiles = int(tile_off[N_GATHER_BLOCKS])

    nc = bacc.Bacc(num_devices=n_cores)
    gx_ext = nc.declare_dram_parameter("gx", [128, tot_tiles * f1], cdt, isOutput=False)
    ind_ext = nc.declare_dram_parameter("ind", [128, tot_slots], cdt, isOutput=False)
    w1_ext = nc.declare_dram_parameter("w1", [f1, f2], cdt, isOutput=False)
    w2_ext = nc.declare_dram_parameter("w2", [f2, f3], cdt, isOutput=False)
    idx_ext = nc.declare_dram_parameter("idxw", [128, tot_slots // 16], i16, isOutput=False)
    cnt_ext = nc.declare_dram_parameter("cnts", [1, max(1, n_calls)], mybir.dt.uint32,
                                        isOutput=False)
    if n_dense:
        ind2_ext = nc.declare_dram_parameter(
            "ind2", [128, n_dense * ns_tiles * 128], cdt, isOutput=False
        )
    out_ext = nc.declare_dram_parameter("out", [NBLK * 128, f3], f32, isOutput=True)

    with tile.TileContext(nc) as tc:
        with tc.tile_pool(name="dram", bufs=1, space="DRAM") as dpool, \
             tc.tile_pool(name="const", bufs=1) as cpool, \
             tc.tile_pool(name="gxp", bufs=3) as gxpool, \
             tc.tile_pool(name="indp", bufs=3) as indpool, \
             tc.tile_pool(name="i2p", bufs=2) as i2pool, \
             tc.tile_pool(name="gbp", bufs=15) as gbpool, \
             tc.tile_pool(name="work", bufs=2) as wpool, \
             tc.tile_pool(name="psagg", bufs=2, space="PSUM") as ps_agg_p, \
             tc.tile_pool(name="pstr", bufs=1, space="PSUM") as ps_tr_p, \
             tc.tile_pool(name="psc1", bufs=1, space="PSUM") as ps_c1_p, \
             tc.tile_pool(name="psh2", bufs=1, space="PSUM") as ps_h2_p, \
             tc.tile_pool(name="pso", bufs=2, space="PSUM") as ps_o_p:

            # ---- DRAM collective buffers (one Shared output per group: a
            # Shared DRAM tensor may only have a single writing instruction)
            cc_in_g = [
                dpool.tile([gnb * 128, f3], cdt, name=f"ccin{g}")
                for g, gnb in enumerate(AG_GROUPS)
            ]
            h2p_g = [
                dpool.tile([n_cores * gnb * 128, f3], cdt, addr_space="Shared",
                           name=f"h2p{g}")
                for g, gnb in enumerate(AG_GROUPS)
            ]
            grp_lo = [0]
            for gnb in AG_GROUPS:
                grp_lo.append(grp_lo[-1] + n_cores * gnb * 128)

            # ---- warm up the collectives engine (ncfw/SPAD startup costs
            # ~35us on the first collective; absorb it before AG(0) is due)
            cc_warm_in = dpool.tile([128, 16], cdt, name="ccwin")
            cc_warm_out = dpool.tile([n_cores * 128, 16], cdt,
                                     addr_space="Shared", name="ccwout")
            cc_warm_sb = cpool.tile([128, 16], cdt, name="ccwsb")
            nc.vector.memset(cc_warm_sb[:, :], 0.0)
            nc.sync.dma_start(out=cc_warm_in[:, :], in_=cc_warm_sb[:, :])
            nc.gpsimd.collective_compute(
                "AllGather",
                mybir.AluOpType.bypass,
                replica_groups=[list(range(n_cores))],
                ins=[cc_warm_in[:, :].opt()],
                outs=[cc_warm_out[:, :].opt()],
            )

            # ---- constants
            cnt_sb = cpool.tile([1, max(1, n_calls)], mybir.dt.uint32)
            nc.sync.dma_start(out=cnt_sb[:, :], in_=cnt_ext[:, :])
            idx_sb = cpool.tile([128, tot_slots // 16], i16)
            nc.sync.dma_start(out=idx_sb[:, :], in_=idx_ext[:, :])
            cnt_reg = nc.gpsimd.to_reg(0)
            call_i = [0]

            # IND tiles of the gather blocks: resident, used by both layers
            indg_sb = cpool.tile([128, gtiles * 128], cdt)
            nc.scalar.dma_start(out=indg_sb[:, :], in_=ind_ext[:, : gtiles * 128])

            w1_sb = cpool.tile([128, kf1 * f2], cdt)  # chunk (k,m) at (k*kf2+m)*128
            for k in range(kf1):
                for m_ in range(kf2):
                    nc.scalar.dma_start(
                        out=w1_sb[:, (k * kf2 + m_) * 128 : (k * kf2 + m_ + 1) * 128],
                        in_=w1_ext[k * 128 : (k + 1) * 128, m_ * 128 : (m_ + 1) * 128],
                    )
            w2_sb = cpool.tile([128, kf2 * f3], cdt)
            for k in range(kf2):
                nc.scalar.dma_start(
                    out=w2_sb[:, k * f3 : (k + 1) * f3],
                    in_=w2_ext[k * 128 : (k + 1) * 128, :],
                )
            ident = cpool.tile([128, 128], cdt)
            make_identity(nc, ident)

            h2f = cpool.tile([128, ns_tiles, f3], cdt, name="h2f")

            # ---- allgather group bookkeeping
            g_of_blk, gb0 = [], []
            acc = 0
            for g, gnb in enumerate(AG_GROUPS):
                for _ in range(gnb):
                    g_of_blk.append(g)
                    gb0.append(acc)
                acc += gnb

            def emit_ag(g):
                nc.gpsimd.collective_compute(
                    "AllGather",
                    mybir.AluOpType.bypass,
                    replica_groups=[list(range(n_cores))],
                    ins=[cc_in_g[g][:, :].opt()],
                    outs=[h2p_g[g][:, :].opt()],
                )
                # land the group into the SBUF-resident H2F tile stack
                t0 = grp_lo[g] // 128
                nt = (grp_lo[g + 1] - grp_lo[g]) // 128
                h2v = h2p_g[g][:, :].rearrange("(s p) f -> p s f", p=128)
                nc.scalar.dma_start(out=h2f[:, t0 : t0 + nt, :], in_=h2v)

            # ---------------- layer 1 ----------------
            # Software-pipelined: block b's aggregation matmuls are emitted
            # BEFORE block b-1's transform so the in-order PE stream never
            # stalls on the transform's vector/scalar steps.
            n_groups = len(AG_GROUPS)
            gb_tiles = {}           # (g, call#) -> (gb2 tile, ct, tg0, done)

            def emit_gathers(gg):
                # layer-2 gather sub-runs of group gg for all gather blocks;
                # dispatched on Pool right after AG(gg), matmuls emitted later
                for b in range(N_GATHER_BLOCKS):
                    tbg = int(t_bg[b, gg])
                    tg0 = int(tile_off_bg[b, gg])
                    done = 0
                    while done < tbg:
                        ct = min(L2_CHUNK, tbg - done)
                        gb2 = gbpool.tile([128, ct, f3], cdt, tag="gbuf")
                        cb = (tg0 + done) * 8
                        if done + ct >= tbg:
                            # final chunk of the sub-run carries the -1 index
                            # tail; skipped rows must read as finite zeros
                            nc.vector.memset(gb2[:, :ct, :], 0.0)
                        nc.gpsimd.reg_load(
                            cnt_reg, cnt_sb[0:1, call_i[0] : call_i[0] + 1]
                        )
                        call_i[0] += 1
                        nc.gpsimd.dma_gather(
                            out_ap=gb2[:, :ct, :],
                            in_ap=h2p_g[gg][:, :],
                            idxs_ap=idx_sb[:, cb : cb + ct * 8],
                            num_idxs=ct * 128,
                            num_idxs_reg=cnt_reg,
                            elem_size=f3,
                        )
                        gb_tiles.setdefault((b, gg), []).append((gb2, ct, tg0 + done))
                        done += ct

            def emit_agg(b):
                tb = t_blocks[b]
                tt0 = int(tile_off[b])
                in_resident = b < N_GATHER_BLOCKS
                ps_agg = ps_agg_p.tile([128, f1], f32, tag="agg")
                done = 0
                while done < tb:
                    ct = min(L1_CHUNK, tb - done)
                    t0 = tt0 + done
                    gxb = gxpool.tile([128, ct, f1], cdt, tag="gx")
                    nc.sync.dma_start(
                        out=gxb[:, :, :],
                        in_=gx_ext[:, t0 * f1 : (t0 + ct) * f1].rearrange(
                            "p (t f) -> p t f", t=ct
                        ),
                    )
                    if in_resident:
                        indb = indg_sb[:, t0 * 128 : (t0 + ct) * 128]
                    else:
                        indb = indpool.tile([128, ct * 128], cdt, tag="ind")
                        nc.scalar.dma_start(
                            out=indb[:, :], in_=ind_ext[:, t0 * 128 : (t0 + ct) * 128]
                        )
                    for t in range(ct):
                        tt = t0 + t
                        nc.tensor.matmul(
                            ps_agg[:, :],
                            lhsT=indb[:, t * 128 : (t + 1) * 128],
                            rhs=gxb[:, t, :],
                            start=(tt == tt0),
                            stop=(tt == tt0 + tb - 1),
                        )
                    done += ct
                return ps_agg

            def emit_xform(b, ps_agg):
                # h2 = relu(agg @ W1) @ W2, then stage into the collective in
                agg_sb = wpool.tile([128, f1], cdt, tag="agg_sb")
                nc.vector.tensor_copy(agg_sb[:, :], ps_agg[:, :])
                ps_tr = ps_tr_p.tile([128, f1], cdt, tag="tr")
                for k in range(kf1):
                    nc.tensor.transpose(
                        ps_tr[:, k * 128 : (k + 1) * 128],
                        agg_sb[:, k * 128 : (k + 1) * 128],
                        ident,
                    )
                aggT_sb = wpool.tile([128, f1], cdt, tag="aggT")
                nc.vector.tensor_copy(aggT_sb[:, :], ps_tr[:, :])

                ps_c1 = ps_c1_p.tile([128, f2], f32, tag="c1")
                firstmm = True
                for m_ in range(kf2):
                    for k in range(kf1):
                        nc.tensor.matmul(
                            ps_c1[:, m_ * 128 : (m_ + 1) * 128],
                            lhsT=w1_sb[:, (k * kf2 + m_) * 128 : (k * kf2 + m_ + 1) * 128],
                            rhs=aggT_sb[:, k * 128 : (k + 1) * 128],
                            start=firstmm,
                            stop=(m_ == kf2 - 1 and k == kf1 - 1),
                        )
                        firstmm = False
                h1T_sb = wpool.tile([128, f2], cdt, tag="h1T")
                nc.scalar.activation(
                    h1T_sb[:, :], ps_c1[:, :], mybir.ActivationFunctionType.Relu
                )
                ps_h2 = ps_h2_p.tile([128, f3], f32, tag="h2")
                for k in range(kf2):
                    nc.tensor.matmul(
                        ps_h2[:, :],
                        lhsT=h1T_sb[:, k * 128 : (k + 1) * 128],
                        rhs=w2_sb[:, k * f3 : (k + 1) * f3],
                        start=(k == 0),
                        stop=(k == kf2 - 1),
                    )
                h2_sb = wpool.tile([128, f3], cdt, tag="h2sb")
                nc.scalar.copy(h2_sb[:, :], ps_h2[:, :])
                g = g_of_blk[b]
                off = (b - gb0[b]) * 128
                nc.sync.dma_start(
                    out=cc_in_g[g][off : off + 128, :], in_=h2_sb[:, :]
                )
                if b == gb0[b] + AG_GROUPS[g] - 1:
                    emit_ag(g)
                    emit_gathers(g)

            pending = None
            for b in range(NBLK):
                ps_agg = emit_agg(b)
                if pending is not None:
                    emit_xform(*pending)
                pending = (b, ps_agg)
            emit_xform(*pending)

            # ---------------- layer 2 ----------------
            # dense blocks, GROUP-MAJOR: all blocks' group-g src tiles run
            # before any group-g+1 tile, so the in-order PE stream never
            # stalls on a later allgather. Partials accumulate in SBUF f32.
            grp_t0 = [grp_lo[g] // 128 for g in range(n_groups)]
            grp_t1 = [grp_lo[g + 1] // 128 for g in range(n_groups)]
            acc_sb = {}
            for bi in range(n_dense):
                acc_sb[bi] = cpool.tile([128, f3], f32, name=f"dacc{bi}")
            for gg in range(n_groups):
                s0, s1 = grp_t0[gg], grp_t1[gg]
                for bi in range(n_dense):
                    done = s0
                    ps_o = ps_o_p.tile([128, f3], f32, tag="o")
                    while done < s1:
                        ct = min(D2_CHUNK, s1 - done)
                        i2 = i2pool.tile([128, ct * 128], cdt, tag="i2")
                        nc.scalar.dma_start(
                            out=i2[:, :],
                            in_=ind2_ext[
                                :, (bi * ns_tiles + done) * 128
                                : (bi * ns_tiles + done + ct) * 128
                            ],
                        )
                        for s in range(ct):
                            nc.tensor.matmul(
                                ps_o[:, :],
                                lhsT=i2[:, s * 128 : (s + 1) * 128],
                                rhs=h2f[:, done + s, :],
                                start=(done + s == s0),
                                stop=(done + s == s1 - 1),
                            )
                        done += ct
                    if gg == 0:
                        nc.vector.tensor_copy(acc_sb[bi][:, :], ps_o[:, :])
                    elif gg < n_groups - 1:
                        nc.vector.tensor_tensor(
                            out=acc_sb[bi][:, :], in0=acc_sb[bi][:, :],
                            in1=ps_o[:, :], op=mybir.AluOpType.add,
                        )
                    else:
                        o_sb = wpool.tile([128, f3], f32, tag="osb")
                        nc.vector.tensor_tensor(
                            out=o_sb[:, :], in0=acc_sb[bi][:, :],
                            in1=ps_o[:, :], op=mybir.AluOpType.add,
                        )
                        b = N_GATHER_BLOCKS + bi
                        nc.sync.dma_start(
                            out=out_ext[b * 128 : (b + 1) * 128, :],
                            in_=o_sb[:, :],
                        )

            # gather blocks: matmuls over the tiles dma_gather'd during the
            # allgather chain (Pool already finished or is finishing them)
            for b in range(N_GATHER_BLOCKS):
                tb = t_blocks[b]
                tt0 = int(tile_off[b])
                ps_o = ps_o_p.tile([128, f3], f32, tag="o")
                for gg in range(n_groups):
                    for gb2, ct, tstart in gb_tiles.get((b, gg), []):
                        for t in range(ct):
                            tt = tstart + t
                            nc.tensor.matmul(
                                ps_o[:, :],
                                lhsT=indg_sb[:, tt * 128 : (tt + 1) * 128],
                                rhs=gb2[:, t, :],
                                start=(tt == tt0),
                                stop=(tt == tt0 + tb - 1),
                            )
                o_sb = wpool.tile([128, f3], f32, tag="osb")
                nc.scalar.copy(o_sb[:, :], ps_o[:, :])
                nc.sync.dma_start(
                    out=out_ext[b * 128 : (b + 1) * 128, :], in_=o_sb[:, :]
                )

    nc.finalize()
    return nc


# --------------------------------------------------------------------------
# top level
# --------------------------------------------------------------------------
def build_all(x, edge_index, edge_weight, W1, W2, n_cores=N_CORES,
              compute_dtype=COMPUTE_DTYPE):
    if compute_dtype == "bf16":
        import ml_dtypes

        np_cdt = ml_dtypes.bfloat16
    else:
        np_cdt = np.float32
    W1c = np.ascontiguousarray(np.asarray(W1, dtype=np.float32).astype(np_cdt))
    W2c = np.ascontiguousarray(np.asarray(W2, dtype=np.float32).astype(np_cdt))
    n_nodes = np.asarray(x).shape[0]
    f1, f2, f3 = W1c.shape[0], W1c.shape[1], W2c.shape[1]
    g = _pack_graph(x, edge_index, edge_weight, n_nodes, n_cores, np_cdt)
    nc = _build_nc(
        f1, f2, f3, g["t_blocks"], g["tile_off"], n_cores, g["n_calls"],
        g["ns_tiles"], g["n_pos"], g["t_bg"], g["tile_off_bg"],
        compute_dtype=compute_dtype,
    )
    in_maps = []
    for c in range(n_cores):
        in_maps.append({
            "gx": g["gx"][c],
            "ind": g["ind"][c],
            "w1": W1c,
            "w2": W2c,
            "idxw": g["idxw"][c],
            "cnts": g["cnts"][c],
            "ind2": g["ind2"][c],
        })
    return nc, in_maps, g


def _unpermute(res, g, n_nodes, f3, n_cores):
    out = np.empty((n_nodes, f3), dtype=np.float32)
    bin_nodes = g["bin_nodes"]
    for c in range(n_cores):
        oc = np.asarray(res[c])            # [NBLK*128, f3]
        for b in range(NBLK):
            nodes = bin_nodes[c * NBLK + b]
            out[nodes] = oc[b * 128 : b * 128 + BIN_CAP]
    return out


def kernel(x, edge_index, edge_weight, W1, W2):
    from concourse.bass_utils import run_bass_kernel_spmd

    nc, in_maps, g = build_all(x, edge_index, edge_weight, W1, W2)
    res = run_bass_kernel_spmd(nc, in_maps, list(range(N_CORES)))
    outs = [res.results[c]["out"] for c in range(N_CORES)]
    return _unpermute(outs, g, np.asarray(x).shape[0], outs[0].shape[1], N_CORES)
